# revision 12
# baseline (speedup 1.0000x reference)
"""Trainium2 Bass kernel for nn_Dist_Loss (segment_reduce).

Data-parallel over 8 NeuronCores: each core takes 1/8 of the rows of
feat1/feat2/label1/label2, computes local per-class sums+counts via one-hot
matmuls (PE, PSUM-accumulated), AllGathers the tiny [2*C, D+1] stats to form
global class centers, then computes per-row distances to own-class centers
entirely from SBUF-resident features (single HBM pass).  Per-class distance
sums are reduced on-device; the final scalar loss formula (tiny, O(C^2 * D))
runs on the host in numpy float32, which reproduces the reference's fp32
overflow semantics (the loss is +inf for the reference inputs).

Features and one-hots are held in bf16 on-chip (one-hots are exact in bf16;
matmuls accumulate fp32 in PSUM) so the PE avoids the 2-pass fp32 LOW_HIGH
matmul mode and LDWEIGHTS uses fast weight load.

Layout: sample s = p*NCOL + n  (p = SBUF partition, n = sample-column).
Per group of GROUP=8 sample-columns (1024 samples):
  - segment sums:  one matmul  lhsT=O_g [128, 80], rhs=feat_g [128, 512]
    -> psum [80, 512]; the 8 diagonal [10, 64] blocks hold valid partial
    sums; the diagonal is folded with a masked matmul afterwards.
  - distances:     per double-group [128, 1024] PSUM: a -I matmul writes
    -feat, a gather matmul (lhsT = PE-transposed one-hot [80, 128],
    rhs = block-diag centers [80, 512]) accumulates +center[label];
    ACT squares, DVE reduces each 64-block -> per-row d^2.

Class index convention: rows 0..9 = domain 1, rows 10..19 = domain 2.
"""

import numpy as np

try:
    import concourse.bass as bass
except ImportError:  # pragma: no cover - fallback when PYTHONPATH is missing
    import sys

    sys.path.insert(0, "/opt/trn_rl_repo")
    import concourse.bass as bass

import concourse.bacc as bacc
import concourse.mybir as mybir
from concourse import tile
from concourse.bass_utils import run_bass_kernel_spmd

F32 = mybir.dt.float32
BF16 = mybir.dt.bfloat16
I32 = mybir.dt.int32
ALU = mybir.AluOpType
AF = mybir.ActivationFunctionType
AX = mybir.AxisListType

N_CORES = 8
N_GLOBAL = 262144
D = 64
C = 10
P = 128
GROUP = 8    # sample-columns per matmul ([*, 512] = one PSUM bank row)
GROUP2 = 16  # sample-columns per pass-B psum tile ([128, 1024], 2 banks)


def _build_nc(n_loc: int, n_cores: int):
    """Trace the SPMD kernel for a per-core shard of n_loc rows per domain."""
    assert n_loc % (P * GROUP2) == 0
    ncol = n_loc // P            # sample-columns per partition, per domain
    ngrp = ncol // GROUP
    ngrp2 = ncol // GROUP2
    nchunks = max(1, ncol // 32)  # DMA chunks per domain
    ccols = ncol // nchunks       # sample-columns per chunk
    assert ccols % GROUP2 == 0

    nc = bacc.Bacc(None, num_devices=n_cores)

    feat_in = [
        nc.dram_tensor(f"feat{d + 1}", [n_loc, D], F32, kind="ExternalInput")
        for d in range(2)
    ]
    lab_in = [
        nc.dram_tensor(f"label{d + 1}", [n_loc], I32, kind="ExternalInput")
        for d in range(2)
    ]
    out_dram = nc.dram_tensor("out", [2 * C, D + 2], F32, kind="ExternalOutput")
    ag_in = nc.dram_tensor("ag_in", [2 * C, D + 1], F32)
    ag_out = nc.dram_tensor(
        "ag_out",
        [2 * C * n_cores, D + 1],
        F32,
        addr_space="Shared" if n_cores > 4 else "Local",
    )

    with tile.TileContext(nc) as tc:
        with (
            tc.tile_pool(name="big", bufs=1) as big,
            tc.tile_pool(name="ring", bufs=3) as ring,
            tc.tile_pool(name="ps", bufs=1, space="PSUM") as pp,
            tc.tile_pool(name="psr", bufs=2, space="PSUM") as psr,
        ):
            # ---------------- constants ----------------
            iota10 = big.tile([P, C], I32, tag="iota10")
            nc.gpsimd.iota(iota10[:], pattern=[[1, C]], base=0, channel_multiplier=0)

            iota_pj = big.tile([P, P], I32, tag="iota_pj")
            nc.gpsimd.iota(
                iota_pj[:], pattern=[[-1, P]], base=0, channel_multiplier=1
            )
            ident = big.tile([P, P], F32, tag="ident")
            nc.vector.tensor_scalar(
                out=ident[:], in0=iota_pj[:], scalar1=0, scalar2=None, op0=ALU.is_equal
            )
            identb = big.tile([P, P], BF16, tag="identb")
            nc.vector.tensor_copy(identb[:], ident[:])
            negi = big.tile([P, P], BF16, tag="negi")
            nc.vector.tensor_scalar(
                out=negi[:],
                in0=iota_pj[:],
                scalar1=0,
                scalar2=-1.0,
                op0=ALU.is_equal,
                op1=ALU.mult,
            )
            ones = big.tile([P, 1], F32, tag="ones")
            nc.vector.memset(ones[:], 1.0)

            # sel0 [10, 80]: sel0[k, 10t+c] = (k == c)
            iota_ki = big.tile([P, 1], I32, tag="iota_ki")
            nc.gpsimd.iota(iota_ki[:], pattern=[[0, 1]], base=0, channel_multiplier=1)
            iota_k = big.tile([P, 1], F32, tag="iota_k")
            nc.vector.tensor_copy(iota_k[:], iota_ki[:])
            itc = big.tile([C, GROUP * C], I32, tag="iota_tc")
            nc.gpsimd.iota(
                itc[:], pattern=[[0, GROUP], [1, C]], base=0, channel_multiplier=0
            )
            sel0 = big.tile([C, GROUP * C], F32, tag="sel0")
            nc.vector.tensor_scalar(
                out=sel0[:], in0=itc[:], scalar1=iota_k[0:C, :], scalar2=None,
                op0=ALU.is_equal,
            )

            # SEL1 [80, 10] = sel0.T, via PE transpose (folds sums diagonal)
            sel1 = big.tile([GROUP * C, C], F32, tag="sel1")
            sel1_ps = psr.tile([GROUP * C, P], F32, tag="gps", name="sel1_ps")
            nc.tensor.transpose(sel1_ps[:, 0:C], sel0[:], ident[0:C, 0:C])
            nc.scalar.copy(out=sel1[:], in_=sel1_ps[:, 0:C])

            # block-diagonal mask [80, 512]: mask[10t+c, 64t'+j] = (t == t')
            bd_a = big.tile([GROUP, GROUP * C], F32, tag="bd_a")
            bd_ai = big.tile([GROUP, GROUP * C], I32, tag="bd_ai")
            nc.gpsimd.iota(
                bd_ai[:], pattern=[[1, GROUP], [0, C]], base=0, channel_multiplier=0
            )
            nc.vector.tensor_scalar(
                out=bd_a[:], in0=bd_ai[:], scalar1=iota_k[0:GROUP, :],
                scalar2=None, op0=ALU.is_equal,
            )
            bd_b = big.tile([GROUP, GROUP * D], F32, tag="bd_b")
            bd_bi = big.tile([GROUP, GROUP * D], I32, tag="bd_bi")
            nc.gpsimd.iota(
                bd_bi[:], pattern=[[1, GROUP], [0, D]], base=0, channel_multiplier=0
            )
            nc.vector.tensor_scalar(
                out=bd_b[:], in0=bd_bi[:], scalar1=iota_k[0:GROUP, :],
                scalar2=None, op0=ALU.is_equal,
            )
            bdmask = big.tile([GROUP * C, GROUP * D], F32, tag="bdmask")
            bd_ps = psr.tile([GROUP * C, GROUP * D], F32, tag="gps", name="bd_ps")
            nc.tensor.matmul(bd_ps[:], bd_a[:], bd_b[:], start=True, stop=True)
            nc.scalar.copy(out=bdmask[:], in_=bd_ps[:])

            # ---------------- label load + one-hot build ----------------
            lab_sb = []
            o_all = []
            ot_all = []
            cnt_pp = []
            for d in range(2):
                lab = big.tile([P, ncol], I32, tag=f"lab{d}")
                nc.sync.dma_start(
                    out=lab[:], in_=lab_in[d][:].rearrange("(p n) -> p n", p=P)
                )
                lab_sb.append(lab)

                oa = big.tile([P, ncol * C], BF16, tag=f"oall{d}")
                nc.vector.tensor_tensor(
                    out=oa[:].rearrange("p (n c) -> p n c", c=C),
                    in0=lab[:].unsqueeze(2).broadcast_to([P, ncol, C]),
                    in1=iota10[:].unsqueeze(1).broadcast_to([P, ncol, C]),
                    op=ALU.is_equal,
                )
                o_all.append(oa)

                # per-partition class counts, folded across partitions by MM
                cp = big.tile([P, C], F32, tag=f"cntpp{d}")
                nc.vector.tensor_reduce(
                    out=cp[:],
                    in_=oa[:].rearrange("p (n c) -> p c n", c=C),
                    axis=AX.X,
                    op=ALU.add,
                )
                cnt_pp.append(cp)

                # transposed one-hots (bf16, exact) for pass-B gathers
                ota = big.tile([GROUP * C, ngrp * P], BF16, tag=f"otall{d}")
                for g in range(ngrp):
                    otp = psr.tile([GROUP * C, P], BF16, tag="gps",
                                   name=f"otp{d}_{g}")
                    nc.tensor.transpose(
                        otp[:],
                        oa[:, g * GROUP * C : (g + 1) * GROUP * C],
                        identb[:],
                    )
                    nc.scalar.copy(out=ota[:, g * P : (g + 1) * P], in_=otp[:])
                ot_all.append(ota)

            # ---------------- feature load (bf16 cast) + segment sums ----
            # psum_cm columns: [counts_d1, md_d1, counts_d2, md_d2]
            psum_cm = pp.tile([C, 4], F32, tag="cm")
            for d in range(2):
                nc.tensor.matmul(
                    psum_cm[:, 2 * d : 2 * d + 1],
                    cnt_pp[d][:],
                    ones[:],
                    start=True,
                    stop=True,
                    skip_group_check=True,
                )

            feat_sb = [[None] * nchunks for _ in range(2)]
            for d in range(2):
                fr = feat_in[d][:].rearrange("(p n) d -> p (n d)", p=P)
                for k in range(nchunks):
                    ft = big.tile([P, ccols * D], BF16, tag=f"feat{d}_{k}",
                                  name=f"feat{d}_{k}")
                    nc.gpsimd.dma_start(
                        out=ft[:], in_=fr[:, k * ccols * D : (k + 1) * ccols * D]
                    )
                    feat_sb[d][k] = ft

            psum_sums = [
                pp.tile([GROUP * C, GROUP * D], F32, tag=f"sums{d}", name=f"sums{d}")
                for d in range(2)
            ]
            for d in range(2):
                for g in range(ngrp):
                    k = (g * GROUP) // ccols
                    c0 = (g * GROUP) % ccols
                    nc.tensor.matmul(
                        psum_sums[d][:],
                        o_all[d][:, g * GROUP * C : (g + 1) * GROUP * C],
                        feat_sb[d][k][:, c0 * D : (c0 + GROUP) * D],
                        start=(g == 0),
                        stop=(g == ngrp - 1),
                    )

            # fold the block-diagonal: sums[c, j] = sum_t psum[10t+c, 64t+j]
            stats_sb = []
            for d in range(2):
                s_sb = ring.tile([GROUP * C, GROUP * D], F32, tag="d2r",
                                 name=f"ssb{d}")
                nc.scalar.copy(out=s_sb[:], in_=psum_sums[d][:])
                nc.vector.tensor_tensor(
                    out=s_sb[:], in0=s_sb[:], in1=bdmask[:], op=ALU.mult
                )
                fold_ps = pp.tile([C, GROUP * D], F32, tag="foldps",
                                  name=f"foldps{d}")
                nc.tensor.matmul(fold_ps[:], sel1[:], s_sb[:], start=True, stop=True)
                st = big.tile([C, D + 1], F32, tag=f"stats{d}", name=f"stats{d}")
                nc.vector.tensor_reduce(
                    out=st[:, 0:D],
                    in_=fold_ps[:].rearrange("c (t e) -> c e t", t=GROUP),
                    axis=AX.X,
                    op=ALU.add,
                )
                nc.vector.tensor_copy(
                    st[:, D : D + 1], psum_cm[:, 2 * d : 2 * d + 1]
                )
                stats_sb.append(st)

            # ---------------- AllGather stats + centers ----------------
            agv = ag_in[:].rearrange("(d c) e -> d c e", d=2)
            for d in range(2):
                nc.sync.dma_start(out=agv[d], in_=stats_sb[d][:])
            nc.gpsimd.collective_compute(
                "AllGather",
                ALU.bypass,
                replica_groups=[list(range(n_cores))],
                ins=[ag_in[:].opt()],
                outs=[ag_out[:].opt()],
            )
            # gather as [class=10, (domain, rank, el)]: all APs at base 0
            gath = big.tile([C, 2 * n_cores * (D + 1)], F32, tag="gath")
            gv = gath[:].rearrange("c (d r e) -> c d r e", d=2, r=n_cores)
            sv = ag_out[:].rearrange("(r d c) e -> d c r e", d=2, c=C)
            for d in range(2):
                nc.sync.dma_start(out=gv[:, d], in_=sv[d])
            stats_g = big.tile([C, 2 * (D + 1)], F32, tag="statsg")
            nc.vector.tensor_reduce(
                out=stats_g[:],
                in_=gath[:].rearrange("c (d r e) -> c d e r", d=2, r=n_cores),
                axis=AX.X,
                op=ALU.add,
            )
            cbd = []
            for d in range(2):
                sg = stats_g[:, d * (D + 1) : (d + 1) * (D + 1)]
                maxc = big.tile([C, 1], F32, tag=f"maxc{d}", name=f"maxc{d}")
                nc.vector.tensor_scalar(
                    out=maxc[:], in0=sg[:, D : D + 1], scalar1=1.0,
                    scalar2=None, op0=ALU.max,
                )
                rec = big.tile([C, 1], F32, tag=f"rec{d}", name=f"rec{d}")
                nc.vector.reciprocal(rec[:], maxc[:])
                cen = big.tile([C, D], F32, tag=f"centers{d}", name=f"centers{d}")
                nc.vector.tensor_scalar(
                    out=cen[:], in0=sg[:, 0:D], scalar1=rec[:],
                    scalar2=None, op0=ALU.mult,
                )
                cen_rep = big.tile([C, GROUP * D], F32, tag=f"cenrep{d}",
                                   name=f"cenrep{d}")
                nc.vector.tensor_copy(
                    cen_rep[:].rearrange("c (t e) -> c t e", t=GROUP),
                    cen[:].unsqueeze(1).broadcast_to([C, GROUP, D]),
                )
                # replicate to [80, 512] on PE, then mask to block-diagonal
                cps = psr.tile([GROUP * C, GROUP * D], F32, tag="gps",
                               name=f"cps{d}")
                nc.tensor.matmul(
                    cps[:], sel0[:], cen_rep[:], start=True, stop=True
                )
                cb = big.tile([GROUP * C, GROUP * D], BF16, tag=f"cbd{d}",
                              name=f"cbd{d}")
                nc.vector.tensor_tensor(
                    out=cb[:], in0=cps[:], in1=bdmask[:], op=ALU.mult
                )
                cbd.append(cb)

            # ---------------- pass B: per-row distances ----------------
            d2_all = [big.tile([P, ncol], F32, tag=f"d2all{d}", name=f"d2all{d}")
                      for d in range(2)]
            d_all = [big.tile([P, ncol], BF16, tag=f"dall{d}", name=f"dall{d}")
                     for d in range(2)]
            for d in range(2):
                for g in range(ngrp2):
                    gps = psr.tile([P, GROUP2 * D], F32, tag="gps",
                                   name=f"gps{d}_{g}")
                    k = (g * GROUP2) // ccols
                    c0 = (g * GROUP2) % ccols
                    for h in range(GROUP2 // GROUP):
                        half = gps[:, h * GROUP * D : (h + 1) * GROUP * D]
                        nc.tensor.matmul(
                            half,
                            negi[:],
                            feat_sb[d][k][
                                :, (c0 + h * GROUP) * D : (c0 + (h + 1) * GROUP) * D
                            ],
                            start=True,
                            stop=False,
                        )
                        gg = g * (GROUP2 // GROUP) + h
                        nc.tensor.matmul(
                            half,
                            ot_all[d][:, gg * P : (gg + 1) * P],
                            cbd[d][:],
                            start=False,
                            stop=True,
                        )
                    d2r = ring.tile([P, GROUP2 * D], F32, tag="d2r",
                                    name=f"d2r{d}_{g}")
                    nc.scalar.activation(d2r[:], gps[:], AF.Square)
                    nc.vector.tensor_reduce(
                        out=d2_all[d][:, g * GROUP2 : (g + 1) * GROUP2],
                        in_=d2r[:].rearrange("p (t e) -> p t e", e=D),
                        axis=AX.X,
                        op=ALU.add,
                    )
                nc.scalar.activation(d_all[d][:], d2_all[d][:], AF.Sqrt)

                # per-class distance sums: W = O * d, fold n then partitions
                w_all = big.tile([P, ncol * C], BF16, tag=f"wall{d}",
                                 name=f"wall{d}")
                nc.vector.tensor_tensor(
                    out=w_all[:].rearrange("p (n c) -> p n c", c=C),
                    in0=o_all[d][:].rearrange("p (n c) -> p n c", c=C),
                    in1=d_all[d][:].unsqueeze(2).broadcast_to([P, ncol, C]),
                    op=ALU.mult,
                )
                mv = big.tile([P, C], F32, tag=f"mv{d}", name=f"mv{d}")
                nc.vector.tensor_reduce(
                    out=mv[:],
                    in_=w_all[:].rearrange("p (n c) -> p c n", c=C),
                    axis=AX.X,
                    op=ALU.add,
                )
                nc.tensor.matmul(
                    psum_cm[:, 2 * d + 1 : 2 * d + 2],
                    mv[:],
                    ones[:],
                    start=True,
                    stop=True,
                    skip_group_check=True,
                )

            # ---------------- output ----------------
            odv = out_dram[:].rearrange("(d c) e -> d c e", d=2)
            for d in range(2):
                osb = big.tile([C, D + 2], F32, tag=f"outsb{d}", name=f"outsb{d}")
                nc.vector.tensor_copy(
                    osb[:, 0 : D + 1],
                    stats_g[:, d * (D + 1) : (d + 1) * (D + 1)],
                )
                nc.vector.tensor_copy(
                    osb[:, D + 1 : D + 2], psum_cm[:, 2 * d + 1 : 2 * d + 2]
                )
                nc.sync.dma_start(out=odv[d], in_=osb[:])

    nc.compile()
    return nc


_NC_CACHE = {}


def _get_nc(n_loc, n_cores):
    key = (n_loc, n_cores)
    if key not in _NC_CACHE:
        _NC_CACHE[key] = _build_nc(n_loc, n_cores)
    return _NC_CACHE[key]


def host_epilogue(stats_g: np.ndarray, md: np.ndarray) -> np.float32:
    """Final scalar loss from global stats ([20, 65]) and md sums ([20]).

    Pure numpy float32; mirrors the reference formula exactly (including the
    fp32 exp overflow -> inf behavior).
    """
    f32 = np.float32
    sums = stats_g[:, :D].astype(f32)
    counts = stats_g[:, D].astype(f32)
    maxc = np.maximum(counts, f32(1.0))
    centers = sums / maxc[:, None]
    m = (md.astype(f32) / maxc).astype(f32)

    c1, c2 = counts[:C], counts[C:]
    valid_intra = (c1 > 1.0) & (c2 > 1.0)
    intra = f32(np.sum(np.where(valid_intra, m[:C] + m[C:], f32(0.0)), dtype=f32))

    ctr1, ctr2 = centers[:C], centers[C:]
    diff = ctr1[:, None, :] - ctr2[None, :, :]
    pd = np.sqrt(np.sum(diff * diff, axis=-1, dtype=f32)).astype(f32)
    valid_c = (c1 > 0.0) & (c2 > 0.0)
    w = (valid_c[:, None] & valid_c[None, :]).astype(f32)
    n_valid = f32(np.sum(valid_c.astype(f32), dtype=f32))
    if n_valid > 1.0:
        inter = f32(np.sum(pd * w, dtype=f32) / np.maximum(n_valid * n_valid, f32(1.0)))
    else:
        inter = f32(0.0)

    normalized = f32(intra / (inter + f32(1e-8)))
    if inter > 0.0:
        with np.errstate(over="ignore"):
            loss = f32(np.log1p(np.exp(normalized / f32(10.0), dtype=f32), dtype=f32))
    else:
        loss = intra
    return np.float32(loss)


def kernel(feat1, label1, feat2, label2, _n_cores=N_CORES, _trace=False):
    n = feat1.shape[0]
    n_loc = n // _n_cores
    nc = _get_nc(n_loc, _n_cores)

    in_maps = []
    for i in range(_n_cores):
        s = slice(i * n_loc, (i + 1) * n_loc)
        in_maps.append(
            {
                "feat1": np.ascontiguousarray(feat1[s]),
                "label1": np.ascontiguousarray(label1[s]),
                "feat2": np.ascontiguousarray(feat2[s]),
                "label2": np.ascontiguousarray(label2[s]),
            }
        )

    res = run_bass_kernel_spmd(
        nc, in_maps, core_ids=list(range(_n_cores)), trace=_trace
    )
    outs = [r["out"] for r in res.results]
    stats_g = outs[0][:, : D + 1]
    md = np.sum([o[:, D + 1] for o in outs], axis=0, dtype=np.float32)
    loss = host_epilogue(stats_g, md)
    if _trace:
        kernel.last_exec_time_ns = res.exec_time_ns
    return loss


kernel.last_exec_time_ns = None


# revision 13
# speedup vs baseline: 1.0642x; 1.0642x over previous
"""Trainium2 Bass kernel for nn_Dist_Loss (segment_reduce).

Data-parallel over 8 NeuronCores: each core takes 1/8 of the rows of
feat1/feat2/label1/label2, computes local per-class sums+counts via one-hot
matmuls (PE, PSUM-accumulated), AllGathers the tiny [2*C, D+1] stats to form
global class centers, then computes per-row distances to own-class centers
entirely from SBUF-resident features (single HBM pass).  Per-class distance
sums are reduced on-device; the final scalar loss formula (tiny, O(C^2 * D))
runs on the host in numpy float32, which reproduces the reference's fp32
overflow semantics (the loss is +inf for the reference inputs).

Features and one-hots are held in bf16 on-chip (one-hots are exact in bf16;
matmuls accumulate fp32 in PSUM) so the PE avoids the 2-pass fp32 LOW_HIGH
matmul mode and LDWEIGHTS uses fast weight load.

Layout: sample s = p*NCOL + n  (p = SBUF partition, n = sample-column).
Per group of GROUP=8 sample-columns (1024 samples):
  - segment sums:  one matmul  lhsT=O_g [128, 80], rhs=feat_g [128, 512]
    -> psum [80, 512]; the 8 diagonal [10, 64] blocks hold valid partial
    sums; the diagonal is folded with a masked matmul afterwards.
  - distances:     per double-group [128, 1024] PSUM: a -I matmul writes
    -feat, a gather matmul (lhsT = PE-transposed one-hot [80, 128],
    rhs = block-diag centers [80, 512]) accumulates +center[label];
    ACT squares, DVE reduces each 64-block -> per-row d^2.

Class index convention: rows 0..9 = domain 1, rows 10..19 = domain 2.
"""

import numpy as np

try:
    import concourse.bass as bass
except ImportError:  # pragma: no cover - fallback when PYTHONPATH is missing
    import sys

    sys.path.insert(0, "/opt/trn_rl_repo")
    import concourse.bass as bass

import concourse.bacc as bacc
import concourse.mybir as mybir
from concourse import tile
from concourse.bass_utils import run_bass_kernel_spmd

F32 = mybir.dt.float32
BF16 = mybir.dt.bfloat16
I32 = mybir.dt.int32
ALU = mybir.AluOpType
AF = mybir.ActivationFunctionType
AX = mybir.AxisListType

N_CORES = 8
N_GLOBAL = 262144
D = 64
C = 10
P = 128
GROUP = 8    # sample-columns per matmul ([*, 512] = one PSUM bank row)
GROUP2 = 16  # sample-columns per pass-B psum tile ([128, 1024], 2 banks)


def _build_nc(n_loc: int, n_cores: int):
    """Trace the SPMD kernel for a per-core shard of n_loc rows per domain."""
    assert n_loc % (P * GROUP2) == 0
    ncol = n_loc // P            # sample-columns per partition, per domain
    ngrp = ncol // GROUP
    ngrp2 = ncol // GROUP2
    nchunks = max(1, ncol // 32)  # DMA chunks per domain
    ccols = ncol // nchunks       # sample-columns per chunk
    assert ccols % GROUP2 == 0

    nc = bacc.Bacc(None, num_devices=n_cores)

    feat_in = [
        nc.dram_tensor(f"feat{d + 1}", [n_loc, D], F32, kind="ExternalInput")
        for d in range(2)
    ]
    lab_in = [
        nc.dram_tensor(f"label{d + 1}", [n_loc], I32, kind="ExternalInput")
        for d in range(2)
    ]
    out_dram = nc.dram_tensor("out", [2 * C, D + 2], F32, kind="ExternalOutput")
    ag_in = [nc.dram_tensor(f"ag_in{d}", [C, D + 1], F32) for d in range(2)]
    ag_out = [
        nc.dram_tensor(
            f"ag_out{d}",
            [C * n_cores, D + 1],
            F32,
            addr_space="Shared" if n_cores > 4 else "Local",
        )
        for d in range(2)
    ]

    with tile.TileContext(nc) as tc:
        with (
            tc.tile_pool(name="big", bufs=1) as big,
            tc.tile_pool(name="ring", bufs=3) as ring,
            tc.tile_pool(name="ps", bufs=1, space="PSUM") as pp,
            tc.tile_pool(name="psr", bufs=2, space="PSUM") as psr,
        ):
            # ---------------- constants ----------------
            iota10 = big.tile([P, C], I32, tag="iota10")
            nc.gpsimd.iota(iota10[:], pattern=[[1, C]], base=0, channel_multiplier=0)

            iota_pj = big.tile([P, P], I32, tag="iota_pj")
            nc.gpsimd.iota(
                iota_pj[:], pattern=[[-1, P]], base=0, channel_multiplier=1
            )
            ident = big.tile([P, P], F32, tag="ident")
            nc.vector.tensor_scalar(
                out=ident[:], in0=iota_pj[:], scalar1=0, scalar2=None, op0=ALU.is_equal
            )
            identb = big.tile([P, P], BF16, tag="identb")
            nc.vector.tensor_copy(identb[:], ident[:])
            negi = big.tile([P, P], BF16, tag="negi")
            nc.vector.tensor_scalar(
                out=negi[:],
                in0=iota_pj[:],
                scalar1=0,
                scalar2=-1.0,
                op0=ALU.is_equal,
                op1=ALU.mult,
            )
            ones = big.tile([P, 1], F32, tag="ones")
            nc.vector.memset(ones[:], 1.0)

            # sel0 [10, 80]: sel0[k, 10t+c] = (k == c)
            iota_ki = big.tile([P, 1], I32, tag="iota_ki")
            nc.gpsimd.iota(iota_ki[:], pattern=[[0, 1]], base=0, channel_multiplier=1)
            iota_k = big.tile([P, 1], F32, tag="iota_k")
            nc.vector.tensor_copy(iota_k[:], iota_ki[:])
            itc = big.tile([C, GROUP * C], I32, tag="iota_tc")
            nc.gpsimd.iota(
                itc[:], pattern=[[0, GROUP], [1, C]], base=0, channel_multiplier=0
            )
            sel0 = big.tile([C, GROUP * C], F32, tag="sel0")
            nc.vector.tensor_scalar(
                out=sel0[:], in0=itc[:], scalar1=iota_k[0:C, :], scalar2=None,
                op0=ALU.is_equal,
            )

            # SEL1 [80, 10] = sel0.T, via PE transpose (folds sums diagonal)
            sel1 = big.tile([GROUP * C, C], F32, tag="sel1")
            sel1_ps = psr.tile([GROUP * C, P], F32, tag="gps", name="sel1_ps")
            nc.tensor.transpose(sel1_ps[:, 0:C], sel0[:], ident[0:C, 0:C])
            nc.scalar.copy(out=sel1[:], in_=sel1_ps[:, 0:C])

            # block-diagonal mask [80, 512]: mask[10t+c, 64t'+j] = (t == t')
            bd_a = big.tile([GROUP, GROUP * C], F32, tag="bd_a")
            bd_ai = big.tile([GROUP, GROUP * C], I32, tag="bd_ai")
            nc.gpsimd.iota(
                bd_ai[:], pattern=[[1, GROUP], [0, C]], base=0, channel_multiplier=0
            )
            nc.vector.tensor_scalar(
                out=bd_a[:], in0=bd_ai[:], scalar1=iota_k[0:GROUP, :],
                scalar2=None, op0=ALU.is_equal,
            )
            bd_b = big.tile([GROUP, GROUP * D], F32, tag="bd_b")
            bd_bi = big.tile([GROUP, GROUP * D], I32, tag="bd_bi")
            nc.gpsimd.iota(
                bd_bi[:], pattern=[[1, GROUP], [0, D]], base=0, channel_multiplier=0
            )
            nc.vector.tensor_scalar(
                out=bd_b[:], in0=bd_bi[:], scalar1=iota_k[0:GROUP, :],
                scalar2=None, op0=ALU.is_equal,
            )
            bdmask = big.tile([GROUP * C, GROUP * D], F32, tag="bdmask")
            bd_ps = psr.tile([GROUP * C, GROUP * D], F32, tag="gps", name="bd_ps")
            nc.tensor.matmul(bd_ps[:], bd_a[:], bd_b[:], start=True, stop=True)
            nc.scalar.copy(out=bdmask[:], in_=bd_ps[:])

            # ---------------- label load + one-hot build ----------------
            lab_sb = []
            o_all = []
            ot_all = []
            cnt_pp = []
            for d in range(2):
                lab = big.tile([P, ncol], I32, tag=f"lab{d}")
                nc.sync.dma_start(
                    out=lab[:], in_=lab_in[d][:].rearrange("(p n) -> p n", p=P)
                )
                lab_sb.append(lab)

                oa = big.tile([P, ncol * C], BF16, tag=f"oall{d}")
                nc.vector.tensor_tensor(
                    out=oa[:].rearrange("p (n c) -> p n c", c=C),
                    in0=lab[:].unsqueeze(2).broadcast_to([P, ncol, C]),
                    in1=iota10[:].unsqueeze(1).broadcast_to([P, ncol, C]),
                    op=ALU.is_equal,
                )
                o_all.append(oa)

                # per-partition class counts, folded across partitions by MM
                cp = big.tile([P, C], F32, tag=f"cntpp{d}")
                nc.vector.tensor_reduce(
                    out=cp[:],
                    in_=oa[:].rearrange("p (n c) -> p c n", c=C),
                    axis=AX.X,
                    op=ALU.add,
                )
                cnt_pp.append(cp)

                # transposed one-hots (bf16, exact) for pass-B gathers
                ota = big.tile([GROUP * C, ngrp * P], BF16, tag=f"otall{d}")
                for g in range(ngrp):
                    otp = psr.tile([GROUP * C, P], BF16, tag="gps",
                                   name=f"otp{d}_{g}")
                    nc.tensor.transpose(
                        otp[:],
                        oa[:, g * GROUP * C : (g + 1) * GROUP * C],
                        identb[:],
                    )
                    nc.scalar.copy(out=ota[:, g * P : (g + 1) * P], in_=otp[:])
                ot_all.append(ota)

            # ---------------- feature load (bf16 cast) + segment sums ----
            # psum_cm columns: [counts_d1, md_d1, counts_d2, md_d2]
            psum_cm = pp.tile([C, 4], F32, tag="cm")
            for d in range(2):
                nc.tensor.matmul(
                    psum_cm[:, 2 * d : 2 * d + 1],
                    cnt_pp[d][:],
                    ones[:],
                    start=True,
                    stop=True,
                    skip_group_check=True,
                )

            feat_sb = [[None] * nchunks for _ in range(2)]
            for d in range(2):
                fr = feat_in[d][:].rearrange("(p n) d -> p (n d)", p=P)
                for k in range(nchunks):
                    ft = big.tile([P, ccols * D], BF16, tag=f"feat{d}_{k}",
                                  name=f"feat{d}_{k}")
                    nc.gpsimd.dma_start(
                        out=ft[:], in_=fr[:, k * ccols * D : (k + 1) * ccols * D]
                    )
                    feat_sb[d][k] = ft

            psum_sums = [
                pp.tile([GROUP * C, GROUP * D], F32, tag=f"sums{d}", name=f"sums{d}")
                for d in range(2)
            ]
            for d in range(2):
                for g in range(ngrp):
                    k = (g * GROUP) // ccols
                    c0 = (g * GROUP) % ccols
                    nc.tensor.matmul(
                        psum_sums[d][:],
                        o_all[d][:, g * GROUP * C : (g + 1) * GROUP * C],
                        feat_sb[d][k][:, c0 * D : (c0 + GROUP) * D],
                        start=(g == 0),
                        stop=(g == ngrp - 1),
                    )

            # per-domain: fold diagonal -> AllGather -> global centers.
            # Domain 0's AllGather overlaps domain 1's streaming sums;
            # domain 1's AllGather overlaps domain 0's pass B.
            stats_g = []
            cbd = []
            for d in range(2):
                # fold the block-diagonal: sums[c, j] = sum_t psum[10t+c, 64t+j]
                s_sb = ring.tile([GROUP * C, GROUP * D], F32, tag="d2r",
                                 name=f"ssb{d}")
                nc.scalar.copy(out=s_sb[:], in_=psum_sums[d][:])
                nc.vector.tensor_tensor(
                    out=s_sb[:], in0=s_sb[:], in1=bdmask[:], op=ALU.mult
                )
                fold_ps = pp.tile([C, GROUP * D], F32, tag="foldps",
                                  name=f"foldps{d}")
                nc.tensor.matmul(fold_ps[:], sel1[:], s_sb[:], start=True, stop=True)
                st = big.tile([C, D + 1], F32, tag=f"stats{d}", name=f"stats{d}")
                nc.vector.tensor_reduce(
                    out=st[:, 0:D],
                    in_=fold_ps[:].rearrange("c (t e) -> c e t", t=GROUP),
                    axis=AX.X,
                    op=ALU.add,
                )
                nc.vector.tensor_copy(
                    st[:, D : D + 1], psum_cm[:, 2 * d : 2 * d + 1]
                )

                nc.sync.dma_start(out=ag_in[d][:], in_=st[:])
                nc.gpsimd.collective_compute(
                    "AllGather",
                    ALU.bypass,
                    replica_groups=[list(range(n_cores))],
                    ins=[ag_in[d][:].opt()],
                    outs=[ag_out[d][:].opt()],
                )
                gath = big.tile([C, n_cores * (D + 1)], F32, tag=f"gath{d}",
                                name=f"gath{d}")
                nc.sync.dma_start(
                    out=gath[:].rearrange("c (r e) -> c r e", r=n_cores),
                    in_=ag_out[d][:].rearrange("(r c) e -> c r e", c=C),
                )
                sg = big.tile([C, D + 1], F32, tag=f"statsg{d}", name=f"statsg{d}")
                nc.vector.tensor_reduce(
                    out=sg[:],
                    in_=gath[:].rearrange("c (r e) -> c e r", r=n_cores),
                    axis=AX.X,
                    op=ALU.add,
                )
                stats_g.append(sg)

                maxc = big.tile([C, 1], F32, tag=f"maxc{d}", name=f"maxc{d}")
                nc.vector.tensor_scalar(
                    out=maxc[:], in0=sg[:, D : D + 1], scalar1=1.0,
                    scalar2=None, op0=ALU.max,
                )
                rec = big.tile([C, 1], F32, tag=f"rec{d}", name=f"rec{d}")
                nc.vector.reciprocal(rec[:], maxc[:])
                cen = big.tile([C, D], F32, tag=f"centers{d}", name=f"centers{d}")
                nc.vector.tensor_scalar(
                    out=cen[:], in0=sg[:, 0:D], scalar1=rec[:],
                    scalar2=None, op0=ALU.mult,
                )
                cen_rep = big.tile([C, GROUP * D], F32, tag=f"cenrep{d}",
                                   name=f"cenrep{d}")
                nc.vector.tensor_copy(
                    cen_rep[:].rearrange("c (t e) -> c t e", t=GROUP),
                    cen[:].unsqueeze(1).broadcast_to([C, GROUP, D]),
                )
                # replicate to [80, 512] on PE, then mask to block-diagonal
                cps = psr.tile([GROUP * C, GROUP * D], F32, tag="gps",
                               name=f"cps{d}")
                nc.tensor.matmul(
                    cps[:], sel0[:], cen_rep[:], start=True, stop=True
                )
                cb = big.tile([GROUP * C, GROUP * D], BF16, tag=f"cbd{d}",
                              name=f"cbd{d}")
                nc.vector.tensor_tensor(
                    out=cb[:], in0=cps[:], in1=bdmask[:], op=ALU.mult
                )
                cbd.append(cb)

            # ---------------- pass B: per-row distances ----------------
            d2_all = [big.tile([P, ncol], F32, tag=f"d2all{d}", name=f"d2all{d}")
                      for d in range(2)]
            d_all = [big.tile([P, ncol], BF16, tag=f"dall{d}", name=f"dall{d}")
                     for d in range(2)]
            for d in range(2):
                for g in range(ngrp2):
                    gps = psr.tile([P, GROUP2 * D], F32, tag="gps",
                                   name=f"gps{d}_{g}")
                    k = (g * GROUP2) // ccols
                    c0 = (g * GROUP2) % ccols
                    for h in range(GROUP2 // GROUP):
                        half = gps[:, h * GROUP * D : (h + 1) * GROUP * D]
                        nc.tensor.matmul(
                            half,
                            negi[:],
                            feat_sb[d][k][
                                :, (c0 + h * GROUP) * D : (c0 + (h + 1) * GROUP) * D
                            ],
                            start=True,
                            stop=False,
                        )
                        gg = g * (GROUP2 // GROUP) + h
                        nc.tensor.matmul(
                            half,
                            ot_all[d][:, gg * P : (gg + 1) * P],
                            cbd[d][:],
                            start=False,
                            stop=True,
                        )
                    d2r = ring.tile([P, GROUP2 * D], BF16, tag="d2rb",
                                    name=f"d2r{d}_{g}")
                    nc.scalar.activation(d2r[:], gps[:], AF.Square)
                    nc.vector.tensor_reduce(
                        out=d2_all[d][:, g * GROUP2 : (g + 1) * GROUP2],
                        in_=d2r[:].rearrange("p (t e) -> p t e", e=D),
                        axis=AX.X,
                        op=ALU.add,
                    )
                nc.scalar.activation(d_all[d][:], d2_all[d][:], AF.Sqrt)

                # per-class distance sums: W = O * d, fold n then partitions
                w_all = big.tile([P, ncol * C], BF16, tag=f"wall{d}",
                                 name=f"wall{d}")
                nc.gpsimd.tensor_tensor(
                    out=w_all[:].rearrange("p (n c) -> p n c", c=C),
                    in0=o_all[d][:].rearrange("p (n c) -> p n c", c=C),
                    in1=d_all[d][:].unsqueeze(2).broadcast_to([P, ncol, C]),
                    op=ALU.mult,
                )
                mv = big.tile([P, C], F32, tag=f"mv{d}", name=f"mv{d}")
                nc.vector.tensor_reduce(
                    out=mv[:],
                    in_=w_all[:].rearrange("p (n c) -> p c n", c=C),
                    axis=AX.X,
                    op=ALU.add,
                )
                nc.tensor.matmul(
                    psum_cm[:, 2 * d + 1 : 2 * d + 2],
                    mv[:],
                    ones[:],
                    start=True,
                    stop=True,
                    skip_group_check=True,
                )

            # ---------------- output ----------------
            odv = out_dram[:].rearrange("(d c) e -> d c e", d=2)
            for d in range(2):
                osb = big.tile([C, D + 2], F32, tag=f"outsb{d}", name=f"outsb{d}")
                nc.vector.tensor_copy(osb[:, 0 : D + 1], stats_g[d][:])
                nc.vector.tensor_copy(
                    osb[:, D + 1 : D + 2], psum_cm[:, 2 * d + 1 : 2 * d + 2]
                )
                nc.sync.dma_start(out=odv[d], in_=osb[:])

    nc.compile()
    return nc


_NC_CACHE = {}


def _get_nc(n_loc, n_cores):
    key = (n_loc, n_cores)
    if key not in _NC_CACHE:
        _NC_CACHE[key] = _build_nc(n_loc, n_cores)
    return _NC_CACHE[key]


def host_epilogue(stats_g: np.ndarray, md: np.ndarray) -> np.float32:
    """Final scalar loss from global stats ([20, 65]) and md sums ([20]).

    Pure numpy float32; mirrors the reference formula exactly (including the
    fp32 exp overflow -> inf behavior).
    """
    f32 = np.float32
    sums = stats_g[:, :D].astype(f32)
    counts = stats_g[:, D].astype(f32)
    maxc = np.maximum(counts, f32(1.0))
    centers = sums / maxc[:, None]
    m = (md.astype(f32) / maxc).astype(f32)

    c1, c2 = counts[:C], counts[C:]
    valid_intra = (c1 > 1.0) & (c2 > 1.0)
    intra = f32(np.sum(np.where(valid_intra, m[:C] + m[C:], f32(0.0)), dtype=f32))

    ctr1, ctr2 = centers[:C], centers[C:]
    diff = ctr1[:, None, :] - ctr2[None, :, :]
    pd = np.sqrt(np.sum(diff * diff, axis=-1, dtype=f32)).astype(f32)
    valid_c = (c1 > 0.0) & (c2 > 0.0)
    w = (valid_c[:, None] & valid_c[None, :]).astype(f32)
    n_valid = f32(np.sum(valid_c.astype(f32), dtype=f32))
    if n_valid > 1.0:
        inter = f32(np.sum(pd * w, dtype=f32) / np.maximum(n_valid * n_valid, f32(1.0)))
    else:
        inter = f32(0.0)

    normalized = f32(intra / (inter + f32(1e-8)))
    if inter > 0.0:
        with np.errstate(over="ignore"):
            loss = f32(np.log1p(np.exp(normalized / f32(10.0), dtype=f32), dtype=f32))
    else:
        loss = intra
    return np.float32(loss)


def kernel(feat1, label1, feat2, label2, _n_cores=N_CORES, _trace=False):
    n = feat1.shape[0]
    n_loc = n // _n_cores
    nc = _get_nc(n_loc, _n_cores)

    in_maps = []
    for i in range(_n_cores):
        s = slice(i * n_loc, (i + 1) * n_loc)
        in_maps.append(
            {
                "feat1": np.ascontiguousarray(feat1[s]),
                "label1": np.ascontiguousarray(label1[s]),
                "feat2": np.ascontiguousarray(feat2[s]),
                "label2": np.ascontiguousarray(label2[s]),
            }
        )

    res = run_bass_kernel_spmd(
        nc, in_maps, core_ids=list(range(_n_cores)), trace=_trace
    )
    outs = [r["out"] for r in res.results]
    stats_g = outs[0][:, : D + 1]
    md = np.sum([o[:, D + 1] for o in outs], axis=0, dtype=np.float32)
    loss = host_epilogue(stats_g, md)
    if _trace:
        kernel.last_exec_time_ns = res.exec_time_ns
    return loss


kernel.last_exec_time_ns = None


# revision 15
# speedup vs baseline: 1.0701x; 1.0055x over previous
"""Trainium2 Bass kernel for nn_Dist_Loss (segment_reduce).

Data-parallel over 8 NeuronCores: each core takes 1/8 of the rows of
feat1/feat2/label1/label2, computes local per-class sums+counts via one-hot
matmuls (PE, PSUM-accumulated), AllGathers the tiny [2*C, D+1] stats to form
global class centers, then computes per-row distances to own-class centers
entirely from SBUF-resident features (single HBM pass).  Per-class distance
sums are reduced on-device; the final scalar loss formula (tiny, O(C^2 * D))
runs on the host in numpy float32, which reproduces the reference's fp32
overflow semantics (the loss is +inf for the reference inputs).

Features and one-hots are held in bf16 on-chip (one-hots are exact in bf16;
matmuls accumulate fp32 in PSUM) so the PE avoids the 2-pass fp32 LOW_HIGH
matmul mode and LDWEIGHTS uses fast weight load.

Layout: sample s = p*NCOL + n  (p = SBUF partition, n = sample-column).
Per group of GROUP=8 sample-columns (1024 samples):
  - segment sums:  one matmul  lhsT=O_g [128, 80], rhs=feat_g [128, 512]
    -> psum [80, 512]; the 8 diagonal [10, 64] blocks hold valid partial
    sums; the diagonal is folded with a masked matmul afterwards.
  - distances:     per double-group [128, 1024] PSUM: a -I matmul writes
    -feat, a gather matmul (lhsT = PE-transposed one-hot [80, 128],
    rhs = block-diag centers [80, 512]) accumulates +center[label];
    ACT squares, DVE reduces each 64-block -> per-row d^2.

Class index convention: rows 0..9 = domain 1, rows 10..19 = domain 2.
"""

import numpy as np

try:
    import concourse.bass as bass
except ImportError:  # pragma: no cover - fallback when PYTHONPATH is missing
    import sys

    sys.path.insert(0, "/opt/trn_rl_repo")
    import concourse.bass as bass

import concourse.bacc as bacc
import concourse.mybir as mybir
from concourse import tile
from concourse.bass_utils import run_bass_kernel_spmd

F32 = mybir.dt.float32
BF16 = mybir.dt.bfloat16
I32 = mybir.dt.int32
ALU = mybir.AluOpType
AF = mybir.ActivationFunctionType
AX = mybir.AxisListType

N_CORES = 8
N_GLOBAL = 262144
D = 64
C = 10
P = 128
GROUP = 8    # sample-columns per matmul ([*, 512] = one PSUM bank row)
GROUP2 = 16  # sample-columns per pass-B psum tile ([128, 1024], 2 banks)


def _build_nc(n_loc: int, n_cores: int):
    """Trace the SPMD kernel for a per-core shard of n_loc rows per domain."""
    assert n_loc % (P * GROUP2) == 0
    ncol = n_loc // P            # sample-columns per partition, per domain
    ngrp = ncol // GROUP
    ngrp2 = ncol // GROUP2
    nchunks = max(1, ncol // 32)  # DMA chunks per domain
    ccols = ncol // nchunks       # sample-columns per chunk
    assert ccols % GROUP2 == 0

    nc = bacc.Bacc(None, num_devices=n_cores)

    feat_in = [
        nc.dram_tensor(f"feat{d + 1}", [n_loc, D], F32, kind="ExternalInput")
        for d in range(2)
    ]
    lab_in = [
        nc.dram_tensor(f"label{d + 1}", [n_loc], I32, kind="ExternalInput")
        for d in range(2)
    ]
    out_dram = nc.dram_tensor("out", [2 * C, D + 2], F32, kind="ExternalOutput")
    ag_in = [nc.dram_tensor(f"ag_in{d}", [C, D + 1], F32) for d in range(2)]
    ag_out = [
        nc.dram_tensor(
            f"ag_out{d}",
            [C * n_cores, D + 1],
            F32,
            addr_space="Shared" if n_cores > 4 else "Local",
        )
        for d in range(2)
    ]

    with tile.TileContext(nc) as tc:
        with (
            tc.tile_pool(name="big", bufs=1) as big,
            tc.tile_pool(name="ring", bufs=4) as ring,
            tc.tile_pool(name="ps", bufs=1, space="PSUM") as pp,
            tc.tile_pool(name="psr", bufs=2, space="PSUM") as psr,
        ):
            # ---------------- constants ----------------
            iota10 = big.tile([P, C], I32, tag="iota10")
            nc.gpsimd.iota(iota10[:], pattern=[[1, C]], base=0, channel_multiplier=0)

            iota_pj = big.tile([P, P], I32, tag="iota_pj")
            nc.gpsimd.iota(
                iota_pj[:], pattern=[[-1, P]], base=0, channel_multiplier=1
            )
            ident = big.tile([P, P], F32, tag="ident")
            nc.vector.tensor_scalar(
                out=ident[:], in0=iota_pj[:], scalar1=0, scalar2=None, op0=ALU.is_equal
            )
            identb = big.tile([P, P], BF16, tag="identb")
            nc.vector.tensor_copy(identb[:], ident[:])
            negi = big.tile([P, P], BF16, tag="negi")
            nc.vector.tensor_scalar(
                out=negi[:],
                in0=iota_pj[:],
                scalar1=0,
                scalar2=-1.0,
                op0=ALU.is_equal,
                op1=ALU.mult,
            )
            ones = big.tile([P, 1], F32, tag="ones")
            nc.vector.memset(ones[:], 1.0)

            # sel0 [10, 80]: sel0[k, 10t+c] = (k == c)
            iota_ki = big.tile([P, 1], I32, tag="iota_ki")
            nc.gpsimd.iota(iota_ki[:], pattern=[[0, 1]], base=0, channel_multiplier=1)
            iota_k = big.tile([P, 1], F32, tag="iota_k")
            nc.vector.tensor_copy(iota_k[:], iota_ki[:])
            itc = big.tile([C, GROUP * C], I32, tag="iota_tc")
            nc.gpsimd.iota(
                itc[:], pattern=[[0, GROUP], [1, C]], base=0, channel_multiplier=0
            )
            sel0 = big.tile([C, GROUP * C], F32, tag="sel0")
            nc.vector.tensor_scalar(
                out=sel0[:], in0=itc[:], scalar1=iota_k[0:C, :], scalar2=None,
                op0=ALU.is_equal,
            )

            # SEL1 [80, 10] = sel0.T, via PE transpose (folds sums diagonal)
            sel1 = big.tile([GROUP * C, C], F32, tag="sel1")
            sel1_ps = psr.tile([GROUP * C, P], F32, tag="gps", name="sel1_ps")
            nc.tensor.transpose(sel1_ps[:, 0:C], sel0[:], ident[0:C, 0:C])
            nc.scalar.copy(out=sel1[:], in_=sel1_ps[:, 0:C])

            # block-diagonal mask [80, 512]: mask[10t+c, 64t'+j] = (t == t')
            bd_a = big.tile([GROUP, GROUP * C], F32, tag="bd_a")
            bd_ai = big.tile([GROUP, GROUP * C], I32, tag="bd_ai")
            nc.gpsimd.iota(
                bd_ai[:], pattern=[[1, GROUP], [0, C]], base=0, channel_multiplier=0
            )
            nc.vector.tensor_scalar(
                out=bd_a[:], in0=bd_ai[:], scalar1=iota_k[0:GROUP, :],
                scalar2=None, op0=ALU.is_equal,
            )
            bd_b = big.tile([GROUP, GROUP * D], F32, tag="bd_b")
            bd_bi = big.tile([GROUP, GROUP * D], I32, tag="bd_bi")
            nc.gpsimd.iota(
                bd_bi[:], pattern=[[1, GROUP], [0, D]], base=0, channel_multiplier=0
            )
            nc.vector.tensor_scalar(
                out=bd_b[:], in0=bd_bi[:], scalar1=iota_k[0:GROUP, :],
                scalar2=None, op0=ALU.is_equal,
            )
            bdmask = big.tile([GROUP * C, GROUP * D], F32, tag="bdmask")
            bd_ps = psr.tile([GROUP * C, GROUP * D], F32, tag="gps", name="bd_ps")
            nc.tensor.matmul(bd_ps[:], bd_a[:], bd_b[:], start=True, stop=True)
            nc.scalar.copy(out=bdmask[:], in_=bd_ps[:])

            # ---------------- label load + one-hot build ----------------
            lab_sb = []
            o_all = []
            ot_all = []
            cnt_pp = []
            for d in range(2):
                lab = big.tile([P, ncol], I32, tag=f"lab{d}")
                nc.sync.dma_start(
                    out=lab[:], in_=lab_in[d][:].rearrange("(p n) -> p n", p=P)
                )
                lab_sb.append(lab)

                oa = big.tile([P, ncol * C], BF16, tag=f"oall{d}")
                nc.vector.tensor_tensor(
                    out=oa[:].rearrange("p (n c) -> p n c", c=C),
                    in0=lab[:].unsqueeze(2).broadcast_to([P, ncol, C]),
                    in1=iota10[:].unsqueeze(1).broadcast_to([P, ncol, C]),
                    op=ALU.is_equal,
                )
                o_all.append(oa)

                # per-partition class counts, folded across partitions by MM
                cp = big.tile([P, C], F32, tag=f"cntpp{d}")
                nc.vector.tensor_reduce(
                    out=cp[:],
                    in_=oa[:].rearrange("p (n c) -> p c n", c=C),
                    axis=AX.X,
                    op=ALU.add,
                )
                cnt_pp.append(cp)

                # transposed one-hots (bf16, exact) for pass-B gathers
                ota = big.tile([GROUP * C, ngrp * P], BF16, tag=f"otall{d}")
                for g in range(ngrp):
                    otp = psr.tile([GROUP * C, P], BF16, tag="gps",
                                   name=f"otp{d}_{g}")
                    nc.tensor.transpose(
                        otp[:],
                        oa[:, g * GROUP * C : (g + 1) * GROUP * C],
                        identb[:],
                    )
                    nc.scalar.copy(out=ota[:, g * P : (g + 1) * P], in_=otp[:])
                ot_all.append(ota)

            # ---------------- feature load (bf16 cast) + segment sums ----
            # psum_cm columns: [counts_d1, md_d1, counts_d2, md_d2]
            psum_cm = pp.tile([C, 4], F32, tag="cm")
            for d in range(2):
                nc.tensor.matmul(
                    psum_cm[:, 2 * d : 2 * d + 1],
                    cnt_pp[d][:],
                    ones[:],
                    start=True,
                    stop=True,
                    skip_group_check=True,
                )

            feat_sb = [[None] * nchunks for _ in range(2)]
            for d in range(2):
                fr = feat_in[d][:].rearrange("(p n) d -> p (n d)", p=P)
                for k in range(nchunks):
                    ft = big.tile([P, ccols * D], BF16, tag=f"feat{d}_{k}",
                                  name=f"feat{d}_{k}")
                    nc.gpsimd.dma_start(
                        out=ft[:], in_=fr[:, k * ccols * D : (k + 1) * ccols * D]
                    )
                    feat_sb[d][k] = ft

            psum_sums = [
                pp.tile([GROUP * C, GROUP * D], F32, tag=f"sums{d}", name=f"sums{d}")
                for d in range(2)
            ]
            for d in range(2):
                for g in range(ngrp):
                    k = (g * GROUP) // ccols
                    c0 = (g * GROUP) % ccols
                    nc.tensor.matmul(
                        psum_sums[d][:],
                        o_all[d][:, g * GROUP * C : (g + 1) * GROUP * C],
                        feat_sb[d][k][:, c0 * D : (c0 + GROUP) * D],
                        start=(g == 0),
                        stop=(g == ngrp - 1),
                    )

            # per-domain: fold diagonal -> AllGather -> global centers.
            # Domain 0's AllGather overlaps domain 1's streaming sums;
            # domain 1's AllGather overlaps domain 0's pass B.
            stats_g = []
            cbd = []
            for d in range(2):
                # fold the block-diagonal: sums[c, j] = sum_t psum[10t+c, 64t+j]
                s_sb = ring.tile([GROUP * C, GROUP * D], F32, tag="d2r",
                                 name=f"ssb{d}")
                nc.scalar.copy(out=s_sb[:], in_=psum_sums[d][:])
                nc.vector.tensor_tensor(
                    out=s_sb[:], in0=s_sb[:], in1=bdmask[:], op=ALU.mult
                )
                fold_ps = pp.tile([C, GROUP * D], F32, tag="foldps",
                                  name=f"foldps{d}")
                nc.tensor.matmul(fold_ps[:], sel1[:], s_sb[:], start=True, stop=True)
                st = big.tile([C, D + 1], F32, tag=f"stats{d}", name=f"stats{d}")
                nc.vector.tensor_reduce(
                    out=st[:, 0:D],
                    in_=fold_ps[:].rearrange("c (t e) -> c e t", t=GROUP),
                    axis=AX.X,
                    op=ALU.add,
                )
                nc.vector.tensor_copy(
                    st[:, D : D + 1], psum_cm[:, 2 * d : 2 * d + 1]
                )

                nc.sync.dma_start(out=ag_in[d][:], in_=st[:])
                nc.gpsimd.collective_compute(
                    "AllGather",
                    ALU.bypass,
                    replica_groups=[list(range(n_cores))],
                    ins=[ag_in[d][:].opt()],
                    outs=[ag_out[d][:].opt()],
                )
                gath = big.tile([C, n_cores * (D + 1)], F32, tag=f"gath{d}",
                                name=f"gath{d}")
                nc.sync.dma_start(
                    out=gath[:].rearrange("c (r e) -> c r e", r=n_cores),
                    in_=ag_out[d][:].rearrange("(r c) e -> c r e", c=C),
                )
                sg = big.tile([C, D + 1], F32, tag=f"statsg{d}", name=f"statsg{d}")
                nc.vector.tensor_reduce(
                    out=sg[:],
                    in_=gath[:].rearrange("c (r e) -> c e r", r=n_cores),
                    axis=AX.X,
                    op=ALU.add,
                )
                stats_g.append(sg)

                maxc = big.tile([C, 1], F32, tag=f"maxc{d}", name=f"maxc{d}")
                nc.vector.tensor_scalar(
                    out=maxc[:], in0=sg[:, D : D + 1], scalar1=1.0,
                    scalar2=None, op0=ALU.max,
                )
                rec = big.tile([C, 1], F32, tag=f"rec{d}", name=f"rec{d}")
                nc.vector.reciprocal(rec[:], maxc[:])
                cen = big.tile([C, D], F32, tag=f"centers{d}", name=f"centers{d}")
                nc.vector.tensor_scalar(
                    out=cen[:], in0=sg[:, 0:D], scalar1=rec[:],
                    scalar2=None, op0=ALU.mult,
                )
                cen_rep = big.tile([C, GROUP * D], F32, tag=f"cenrep{d}",
                                   name=f"cenrep{d}")
                nc.vector.tensor_copy(
                    cen_rep[:].rearrange("c (t e) -> c t e", t=GROUP),
                    cen[:].unsqueeze(1).broadcast_to([C, GROUP, D]),
                )
                # replicate to [80, 512] on PE, then mask to block-diagonal
                cps = psr.tile([GROUP * C, GROUP * D], F32, tag="gps",
                               name=f"cps{d}")
                nc.tensor.matmul(
                    cps[:], sel0[:], cen_rep[:], start=True, stop=True
                )
                cb = big.tile([GROUP * C, GROUP * D], BF16, tag=f"cbd{d}",
                              name=f"cbd{d}")
                nc.vector.tensor_tensor(
                    out=cb[:], in0=cps[:], in1=bdmask[:], op=ALU.mult
                )
                cbd.append(cb)

            # ---------------- pass B: per-row distances ----------------
            d2_all = [big.tile([P, ncol], F32, tag=f"d2all{d}", name=f"d2all{d}")
                      for d in range(2)]
            d_all = [big.tile([P, ncol], BF16, tag=f"dall{d}", name=f"dall{d}")
                     for d in range(2)]
            for d in range(2):
                for g in range(ngrp2):
                    gps = psr.tile([P, GROUP2 * D], F32, tag="gps",
                                   name=f"gps{d}_{g}")
                    k = (g * GROUP2) // ccols
                    c0 = (g * GROUP2) % ccols
                    for h in range(GROUP2 // GROUP):
                        half = gps[:, h * GROUP * D : (h + 1) * GROUP * D]
                        nc.tensor.matmul(
                            half,
                            negi[:],
                            feat_sb[d][k][
                                :, (c0 + h * GROUP) * D : (c0 + (h + 1) * GROUP) * D
                            ],
                            start=True,
                            stop=False,
                        )
                        gg = g * (GROUP2 // GROUP) + h
                        nc.tensor.matmul(
                            half,
                            ot_all[d][:, gg * P : (gg + 1) * P],
                            cbd[d][:],
                            start=False,
                            stop=True,
                        )
                    d2r = ring.tile([P, GROUP2 * D], BF16, tag="d2rb",
                                    name=f"d2r{d}_{g}")
                    nc.scalar.activation(d2r[:], gps[:], AF.Square)
                    nc.vector.tensor_reduce(
                        out=d2_all[d][:, g * GROUP2 : (g + 1) * GROUP2],
                        in_=d2r[:].rearrange("p (t e) -> p t e", e=D),
                        axis=AX.X,
                        op=ALU.add,
                    )

                # per-class distance sums: W = O * d, fold n then partitions,
                # pipelined at half-domain granularity
                w_all = big.tile([P, ncol * C], BF16, tag=f"wall{d}",
                                 name=f"wall{d}")
                mv = big.tile([P, 2 * C], F32, tag=f"mv{d}", name=f"mv{d}")
                nh = ncol // 2
                for q in range(2):
                    nc.scalar.activation(
                        d_all[d][:, q * nh : (q + 1) * nh],
                        d2_all[d][:, q * nh : (q + 1) * nh],
                        AF.Sqrt,
                    )
                    wv = w_all[:, q * nh * C : (q + 1) * nh * C]
                    nc.gpsimd.tensor_tensor(
                        out=wv.rearrange("p (n c) -> p n c", c=C),
                        in0=o_all[d][:, q * nh * C : (q + 1) * nh * C].rearrange(
                            "p (n c) -> p n c", c=C
                        ),
                        in1=d_all[d][:, q * nh : (q + 1) * nh]
                        .unsqueeze(2)
                        .broadcast_to([P, nh, C]),
                        op=ALU.mult,
                    )
                    nc.vector.tensor_reduce(
                        out=mv[:, q * C : (q + 1) * C],
                        in_=wv.rearrange("p (n c) -> p c n", c=C),
                        axis=AX.X,
                        op=ALU.add,
                    )
                    nc.tensor.matmul(
                        psum_cm[:, 2 * d + 1 : 2 * d + 2],
                        mv[:, q * C : (q + 1) * C],
                        ones[:],
                        start=(q == 0),
                        stop=(q == 1),
                        skip_group_check=True,
                    )

            # ---------------- output ----------------
            odv = out_dram[:].rearrange("(d c) e -> d c e", d=2)
            for d in range(2):
                osb = big.tile([C, D + 2], F32, tag=f"outsb{d}", name=f"outsb{d}")
                nc.vector.tensor_copy(osb[:, 0 : D + 1], stats_g[d][:])
                nc.vector.tensor_copy(
                    osb[:, D + 1 : D + 2], psum_cm[:, 2 * d + 1 : 2 * d + 2]
                )
                nc.sync.dma_start(out=odv[d], in_=osb[:])

    nc.compile()
    return nc


_NC_CACHE = {}


def _get_nc(n_loc, n_cores):
    key = (n_loc, n_cores)
    if key not in _NC_CACHE:
        _NC_CACHE[key] = _build_nc(n_loc, n_cores)
    return _NC_CACHE[key]


def host_epilogue(stats_g: np.ndarray, md: np.ndarray) -> np.float32:
    """Final scalar loss from global stats ([20, 65]) and md sums ([20]).

    Pure numpy float32; mirrors the reference formula exactly (including the
    fp32 exp overflow -> inf behavior).
    """
    f32 = np.float32
    sums = stats_g[:, :D].astype(f32)
    counts = stats_g[:, D].astype(f32)
    maxc = np.maximum(counts, f32(1.0))
    centers = sums / maxc[:, None]
    m = (md.astype(f32) / maxc).astype(f32)

    c1, c2 = counts[:C], counts[C:]
    valid_intra = (c1 > 1.0) & (c2 > 1.0)
    intra = f32(np.sum(np.where(valid_intra, m[:C] + m[C:], f32(0.0)), dtype=f32))

    ctr1, ctr2 = centers[:C], centers[C:]
    diff = ctr1[:, None, :] - ctr2[None, :, :]
    pd = np.sqrt(np.sum(diff * diff, axis=-1, dtype=f32)).astype(f32)
    valid_c = (c1 > 0.0) & (c2 > 0.0)
    w = (valid_c[:, None] & valid_c[None, :]).astype(f32)
    n_valid = f32(np.sum(valid_c.astype(f32), dtype=f32))
    if n_valid > 1.0:
        inter = f32(np.sum(pd * w, dtype=f32) / np.maximum(n_valid * n_valid, f32(1.0)))
    else:
        inter = f32(0.0)

    normalized = f32(intra / (inter + f32(1e-8)))
    if inter > 0.0:
        with np.errstate(over="ignore"):
            loss = f32(np.log1p(np.exp(normalized / f32(10.0), dtype=f32), dtype=f32))
    else:
        loss = intra
    return np.float32(loss)


def kernel(feat1, label1, feat2, label2, _n_cores=N_CORES, _trace=False):
    n = feat1.shape[0]
    n_loc = n // _n_cores
    nc = _get_nc(n_loc, _n_cores)

    in_maps = []
    for i in range(_n_cores):
        s = slice(i * n_loc, (i + 1) * n_loc)
        in_maps.append(
            {
                "feat1": np.ascontiguousarray(feat1[s]),
                "label1": np.ascontiguousarray(label1[s]),
                "feat2": np.ascontiguousarray(feat2[s]),
                "label2": np.ascontiguousarray(label2[s]),
            }
        )

    res = run_bass_kernel_spmd(
        nc, in_maps, core_ids=list(range(_n_cores)), trace=_trace
    )
    outs = [r["out"] for r in res.results]
    stats_g = outs[0][:, : D + 1]
    md = np.sum([o[:, D + 1] for o in outs], axis=0, dtype=np.float32)
    loss = host_epilogue(stats_g, md)
    if _trace:
        kernel.last_exec_time_ns = res.exec_time_ns
    return loss


kernel.last_exec_time_ns = None


# revision 16
# speedup vs baseline: 1.0722x; 1.0020x over previous
"""Trainium2 Bass kernel for nn_Dist_Loss (segment_reduce).

Data-parallel over 8 NeuronCores: each core takes 1/8 of the rows of
feat1/feat2/label1/label2, computes local per-class sums+counts via one-hot
matmuls (PE, PSUM-accumulated), AllGathers the tiny [2*C, D+1] stats to form
global class centers, then computes per-row distances to own-class centers
entirely from SBUF-resident features (single HBM pass).  Per-class distance
sums are reduced on-device; the final scalar loss formula (tiny, O(C^2 * D))
runs on the host in numpy float32, which reproduces the reference's fp32
overflow semantics (the loss is +inf for the reference inputs).

Features and one-hots are held in bf16 on-chip (one-hots are exact in bf16;
matmuls accumulate fp32 in PSUM) so the PE avoids the 2-pass fp32 LOW_HIGH
matmul mode and LDWEIGHTS uses fast weight load.

Layout: sample s = p*NCOL + n  (p = SBUF partition, n = sample-column).
Per group of GROUP=8 sample-columns (1024 samples):
  - segment sums:  one matmul  lhsT=O_g [128, 80], rhs=feat_g [128, 512]
    -> psum [80, 512]; the 8 diagonal [10, 64] blocks hold valid partial
    sums; the diagonal is folded with a masked matmul afterwards.
  - distances:     per double-group [128, 1024] PSUM: a -I matmul writes
    -feat, a gather matmul (lhsT = PE-transposed one-hot [80, 128],
    rhs = block-diag centers [80, 512]) accumulates +center[label];
    ACT squares, DVE reduces each 64-block -> per-row d^2.

Class index convention: rows 0..9 = domain 1, rows 10..19 = domain 2.
"""

import numpy as np

try:
    import concourse.bass as bass
except ImportError:  # pragma: no cover - fallback when PYTHONPATH is missing
    import sys

    sys.path.insert(0, "/opt/trn_rl_repo")
    import concourse.bass as bass

import concourse.bacc as bacc
import concourse.mybir as mybir
from concourse import tile
from concourse.bass_utils import run_bass_kernel_spmd

F32 = mybir.dt.float32
BF16 = mybir.dt.bfloat16
I32 = mybir.dt.int32
ALU = mybir.AluOpType
AF = mybir.ActivationFunctionType
AX = mybir.AxisListType

N_CORES = 8
N_GLOBAL = 262144
D = 64
C = 10
P = 128
GROUP = 8    # sample-columns per matmul ([*, 512] = one PSUM bank row)
GROUP2 = 16  # sample-columns per pass-B psum tile ([128, 1024], 2 banks)


def _build_nc(n_loc: int, n_cores: int):
    """Trace the SPMD kernel for a per-core shard of n_loc rows per domain."""
    assert n_loc % (P * GROUP2) == 0
    ncol = n_loc // P            # sample-columns per partition, per domain
    ngrp = ncol // GROUP
    ngrp2 = ncol // GROUP2
    nchunks = max(1, ncol // 32)  # DMA chunks per domain
    ccols = ncol // nchunks       # sample-columns per chunk
    assert ccols % GROUP2 == 0

    nc = bacc.Bacc(None, num_devices=n_cores)

    feat_in = [
        nc.dram_tensor(f"feat{d + 1}", [n_loc, D], F32, kind="ExternalInput")
        for d in range(2)
    ]
    lab_in = [
        nc.dram_tensor(f"label{d + 1}", [n_loc], I32, kind="ExternalInput")
        for d in range(2)
    ]
    out_dram = nc.dram_tensor("out", [2 * C, D + 2], F32, kind="ExternalOutput")
    ag_in = [nc.dram_tensor(f"ag_in{d}", [C, D + 1], F32) for d in range(2)]
    ag_out = [
        nc.dram_tensor(
            f"ag_out{d}",
            [C * n_cores, D + 1],
            F32,
            addr_space="Shared" if n_cores > 4 else "Local",
        )
        for d in range(2)
    ]

    with tile.TileContext(nc) as tc:
        with (
            tc.tile_pool(name="big", bufs=1) as big,
            tc.tile_pool(name="ring", bufs=4) as ring,
            tc.tile_pool(name="ps", bufs=1, space="PSUM") as pp,
            tc.tile_pool(name="psr", bufs=2, space="PSUM") as psr,
        ):
            # ---------------- constants ----------------
            iota10 = big.tile([P, C], I32, tag="iota10")
            nc.gpsimd.iota(iota10[:], pattern=[[1, C]], base=0, channel_multiplier=0)

            iota_pj = big.tile([P, P], I32, tag="iota_pj")
            nc.gpsimd.iota(
                iota_pj[:], pattern=[[-1, P]], base=0, channel_multiplier=1
            )
            ident = big.tile([P, P], F32, tag="ident")
            nc.vector.tensor_scalar(
                out=ident[:], in0=iota_pj[:], scalar1=0, scalar2=None, op0=ALU.is_equal
            )
            identb = big.tile([P, P], BF16, tag="identb")
            nc.vector.tensor_copy(identb[:], ident[:])
            negi = big.tile([P, P], BF16, tag="negi")
            nc.vector.tensor_scalar(
                out=negi[:],
                in0=iota_pj[:],
                scalar1=0,
                scalar2=-1.0,
                op0=ALU.is_equal,
                op1=ALU.mult,
            )
            ones = big.tile([P, 1], F32, tag="ones")
            nc.vector.memset(ones[:], 1.0)

            # sel0 [10, 80]: sel0[k, 10t+c] = (k == c)
            iota_ki = big.tile([P, 1], I32, tag="iota_ki")
            nc.gpsimd.iota(iota_ki[:], pattern=[[0, 1]], base=0, channel_multiplier=1)
            iota_k = big.tile([P, 1], F32, tag="iota_k")
            nc.vector.tensor_copy(iota_k[:], iota_ki[:])
            itc = big.tile([C, GROUP * C], I32, tag="iota_tc")
            nc.gpsimd.iota(
                itc[:], pattern=[[0, GROUP], [1, C]], base=0, channel_multiplier=0
            )
            sel0 = big.tile([C, GROUP * C], F32, tag="sel0")
            nc.vector.tensor_scalar(
                out=sel0[:], in0=itc[:], scalar1=iota_k[0:C, :], scalar2=None,
                op0=ALU.is_equal,
            )

            # SEL1 [80, 10] = sel0.T, via PE transpose (folds sums diagonal)
            sel1 = big.tile([GROUP * C, C], F32, tag="sel1")
            sel1_ps = psr.tile([GROUP * C, P], F32, tag="gps", name="sel1_ps")
            nc.tensor.transpose(sel1_ps[:, 0:C], sel0[:], ident[0:C, 0:C])
            nc.scalar.copy(out=sel1[:], in_=sel1_ps[:, 0:C])

            # block-diagonal mask [80, 512]: mask[10t+c, 64t'+j] = (t == t')
            bd_a = big.tile([GROUP, GROUP * C], F32, tag="bd_a")
            bd_ai = big.tile([GROUP, GROUP * C], I32, tag="bd_ai")
            nc.gpsimd.iota(
                bd_ai[:], pattern=[[1, GROUP], [0, C]], base=0, channel_multiplier=0
            )
            nc.vector.tensor_scalar(
                out=bd_a[:], in0=bd_ai[:], scalar1=iota_k[0:GROUP, :],
                scalar2=None, op0=ALU.is_equal,
            )
            bd_b = big.tile([GROUP, GROUP * D], F32, tag="bd_b")
            bd_bi = big.tile([GROUP, GROUP * D], I32, tag="bd_bi")
            nc.gpsimd.iota(
                bd_bi[:], pattern=[[1, GROUP], [0, D]], base=0, channel_multiplier=0
            )
            nc.vector.tensor_scalar(
                out=bd_b[:], in0=bd_bi[:], scalar1=iota_k[0:GROUP, :],
                scalar2=None, op0=ALU.is_equal,
            )
            bdmask = big.tile([GROUP * C, GROUP * D], F32, tag="bdmask")
            bd_ps = psr.tile([GROUP * C, GROUP * D], F32, tag="gps", name="bd_ps")
            nc.tensor.matmul(bd_ps[:], bd_a[:], bd_b[:], start=True, stop=True)
            nc.scalar.copy(out=bdmask[:], in_=bd_ps[:])

            # ---------------- label load + one-hot build ----------------
            lab_sb = []
            o_all = []
            ot_all = []
            cnt_pp = []
            for d in range(2):
                lab = big.tile([P, ncol], I32, tag=f"lab{d}")
                nc.sync.dma_start(
                    out=lab[:], in_=lab_in[d][:].rearrange("(p n) -> p n", p=P)
                )
                lab_sb.append(lab)

                oa = big.tile([P, ncol * C], BF16, tag=f"oall{d}")
                nc.vector.tensor_tensor(
                    out=oa[:].rearrange("p (n c) -> p n c", c=C),
                    in0=lab[:].unsqueeze(2).broadcast_to([P, ncol, C]),
                    in1=iota10[:].unsqueeze(1).broadcast_to([P, ncol, C]),
                    op=ALU.is_equal,
                )
                o_all.append(oa)

                # per-partition class counts, folded across partitions by MM
                cp = big.tile([P, C], F32, tag=f"cntpp{d}")
                nc.vector.tensor_reduce(
                    out=cp[:],
                    in_=oa[:].rearrange("p (n c) -> p c n", c=C),
                    axis=AX.X,
                    op=ALU.add,
                )
                cnt_pp.append(cp)

                # transposed one-hots (bf16, exact) for pass-B gathers
                ota = big.tile([GROUP * C, ngrp * P], BF16, tag=f"otall{d}")
                for g in range(ngrp):
                    otp = psr.tile([GROUP * C, P], BF16, tag="gps",
                                   name=f"otp{d}_{g}")
                    nc.tensor.transpose(
                        otp[:],
                        oa[:, g * GROUP * C : (g + 1) * GROUP * C],
                        identb[:],
                    )
                    nc.scalar.copy(out=ota[:, g * P : (g + 1) * P], in_=otp[:])
                ot_all.append(ota)

            # ---------------- feature load (bf16 cast) + segment sums ----
            # psum_cm columns: [counts_d1, md_d1, counts_d2, md_d2]
            psum_cm = pp.tile([C, 4], F32, tag="cm")
            for d in range(2):
                nc.tensor.matmul(
                    psum_cm[:, 2 * d : 2 * d + 1],
                    cnt_pp[d][:],
                    ones[:],
                    start=True,
                    stop=True,
                    skip_group_check=True,
                )

            feat_sb = [[None] * nchunks for _ in range(2)]
            for d in range(2):
                fr = feat_in[d][:].rearrange("(p n) d -> p (n d)", p=P)
                for k in range(nchunks):
                    ft = big.tile([P, ccols * D], BF16, tag=f"feat{d}_{k}",
                                  name=f"feat{d}_{k}")
                    nc.gpsimd.dma_start(
                        out=ft[:], in_=fr[:, k * ccols * D : (k + 1) * ccols * D]
                    )
                    feat_sb[d][k] = ft

            psum_sums = [
                pp.tile([GROUP * C, GROUP * D], F32, tag=f"sums{d}", name=f"sums{d}")
                for d in range(2)
            ]
            for d in range(2):
                for g in range(ngrp):
                    k = (g * GROUP) // ccols
                    c0 = (g * GROUP) % ccols
                    nc.tensor.matmul(
                        psum_sums[d][:],
                        o_all[d][:, g * GROUP * C : (g + 1) * GROUP * C],
                        feat_sb[d][k][:, c0 * D : (c0 + GROUP) * D],
                        start=(g == 0),
                        stop=(g == ngrp - 1),
                    )

            # per-domain: fold diagonal -> AllGather -> global centers.
            # Domain 0's AllGather overlaps domain 1's streaming sums;
            # domain 1's AllGather overlaps domain 0's pass B.
            stats_g = []
            cbd = []
            for d in range(2):
                # fold the block-diagonal: sums[c, j] = sum_t psum[10t+c, 64t+j]
                s_sb = ring.tile([GROUP * C, GROUP * D], F32, tag="d2r",
                                 name=f"ssb{d}")
                nc.scalar.copy(out=s_sb[:], in_=psum_sums[d][:])
                nc.vector.tensor_tensor(
                    out=s_sb[:], in0=s_sb[:], in1=bdmask[:], op=ALU.mult
                )
                fold_ps = pp.tile([C, GROUP * D], F32, tag="foldps",
                                  name=f"foldps{d}")
                nc.tensor.matmul(fold_ps[:], sel1[:], s_sb[:], start=True, stop=True)
                st = big.tile([C, D + 1], F32, tag=f"stats{d}", name=f"stats{d}")
                nc.vector.tensor_reduce(
                    out=st[:, 0:D],
                    in_=fold_ps[:].rearrange("c (t e) -> c e t", t=GROUP),
                    axis=AX.X,
                    op=ALU.add,
                )
                nc.vector.tensor_copy(
                    st[:, D : D + 1], psum_cm[:, 2 * d : 2 * d + 1]
                )

                nc.sync.dma_start(out=ag_in[d][:], in_=st[:])
                nc.gpsimd.collective_compute(
                    "AllGather",
                    ALU.bypass,
                    replica_groups=[list(range(n_cores))],
                    ins=[ag_in[d][:].opt()],
                    outs=[ag_out[d][:].opt()],
                )
                gath = big.tile([C, n_cores * (D + 1)], F32, tag=f"gath{d}",
                                name=f"gath{d}")
                nc.sync.dma_start(
                    out=gath[:].rearrange("c (r e) -> c r e", r=n_cores),
                    in_=ag_out[d][:].rearrange("(r c) e -> c r e", c=C),
                )
                sg = big.tile([C, D + 1], F32, tag=f"statsg{d}", name=f"statsg{d}")
                nc.vector.tensor_reduce(
                    out=sg[:],
                    in_=gath[:].rearrange("c (r e) -> c e r", r=n_cores),
                    axis=AX.X,
                    op=ALU.add,
                )
                stats_g.append(sg)

                maxc = big.tile([C, 1], F32, tag=f"maxc{d}", name=f"maxc{d}")
                nc.vector.tensor_scalar(
                    out=maxc[:], in0=sg[:, D : D + 1], scalar1=1.0,
                    scalar2=None, op0=ALU.max,
                )
                rec = big.tile([C, 1], F32, tag=f"rec{d}", name=f"rec{d}")
                nc.vector.reciprocal(rec[:], maxc[:])
                cen = big.tile([C, D], F32, tag=f"centers{d}", name=f"centers{d}")
                nc.vector.tensor_scalar(
                    out=cen[:], in0=sg[:, 0:D], scalar1=rec[:],
                    scalar2=None, op0=ALU.mult,
                )
                cen_rep = big.tile([C, GROUP * D], F32, tag=f"cenrep{d}",
                                   name=f"cenrep{d}")
                nc.vector.tensor_copy(
                    cen_rep[:].rearrange("c (t e) -> c t e", t=GROUP),
                    cen[:].unsqueeze(1).broadcast_to([C, GROUP, D]),
                )
                # replicate to [80, 512] on PE, then mask to block-diagonal
                cps = psr.tile([GROUP * C, GROUP * D], F32, tag="gps",
                               name=f"cps{d}")
                nc.tensor.matmul(
                    cps[:], sel0[:], cen_rep[:], start=True, stop=True
                )
                cb = big.tile([GROUP * C, GROUP * D], BF16, tag=f"cbd{d}",
                              name=f"cbd{d}")
                nc.vector.tensor_tensor(
                    out=cb[:], in0=cps[:], in1=bdmask[:], op=ALU.mult
                )
                cbd.append(cb)

            # ---------------- pass B: per-row distances ----------------
            d2_all = [big.tile([P, ncol], F32, tag=f"d2all{d}", name=f"d2all{d}")
                      for d in range(2)]
            d_all = [big.tile([P, ncol], BF16, tag=f"dall{d}", name=f"dall{d}")
                     for d in range(2)]
            for d in range(2):
                for g in range(ngrp2):
                    gps = psr.tile([P, GROUP2 * D], F32, tag="gps",
                                   name=f"gps{d}_{g}")
                    k = (g * GROUP2) // ccols
                    c0 = (g * GROUP2) % ccols
                    for h in range(GROUP2 // GROUP):
                        half = gps[:, h * GROUP * D : (h + 1) * GROUP * D]
                        nc.tensor.matmul(
                            half,
                            negi[:],
                            feat_sb[d][k][
                                :, (c0 + h * GROUP) * D : (c0 + (h + 1) * GROUP) * D
                            ],
                            start=True,
                            stop=False,
                        )
                        gg = g * (GROUP2 // GROUP) + h
                        nc.tensor.matmul(
                            half,
                            ot_all[d][:, gg * P : (gg + 1) * P],
                            cbd[d][:],
                            start=False,
                            stop=True,
                        )
                    d2r = ring.tile([P, GROUP2 * D], F32, tag="d2rb",
                                    name=f"d2r{d}_{g}")
                    nc.scalar.activation(d2r[:], gps[:], AF.Square)
                    nc.vector.tensor_reduce(
                        out=d2_all[d][:, g * GROUP2 : (g + 1) * GROUP2],
                        in_=d2r[:].rearrange("p (t e) -> p t e", e=D),
                        axis=AX.X,
                        op=ALU.add,
                    )

                # per-class distance sums: W = O * d, fold n then partitions,
                # pipelined at half-domain granularity
                w_all = big.tile([P, ncol * C], F32, tag=f"wall{d}",
                                 name=f"wall{d}")
                mv = big.tile([P, 2 * C], F32, tag=f"mv{d}", name=f"mv{d}")
                nh = ncol // 2
                for q in range(2):
                    nc.scalar.activation(
                        d_all[d][:, q * nh : (q + 1) * nh],
                        d2_all[d][:, q * nh : (q + 1) * nh],
                        AF.Sqrt,
                    )
                    wv = w_all[:, q * nh * C : (q + 1) * nh * C]
                    nc.gpsimd.tensor_tensor(
                        out=wv.rearrange("p (n c) -> p n c", c=C),
                        in0=o_all[d][:, q * nh * C : (q + 1) * nh * C].rearrange(
                            "p (n c) -> p n c", c=C
                        ),
                        in1=d_all[d][:, q * nh : (q + 1) * nh]
                        .unsqueeze(2)
                        .broadcast_to([P, nh, C]),
                        op=ALU.mult,
                    )
                    nc.vector.tensor_reduce(
                        out=mv[:, q * C : (q + 1) * C],
                        in_=wv.rearrange("p (n c) -> p c n", c=C),
                        axis=AX.X,
                        op=ALU.add,
                    )
                    nc.tensor.matmul(
                        psum_cm[:, 2 * d + 1 : 2 * d + 2],
                        mv[:, q * C : (q + 1) * C],
                        ones[:],
                        start=(q == 0),
                        stop=(q == 1),
                        skip_group_check=True,
                    )

            # ---------------- output ----------------
            odv = out_dram[:].rearrange("(d c) e -> d c e", d=2)
            for d in range(2):
                osb = big.tile([C, D + 2], F32, tag=f"outsb{d}", name=f"outsb{d}")
                nc.vector.tensor_copy(osb[:, 0 : D + 1], stats_g[d][:])
                nc.vector.tensor_copy(
                    osb[:, D + 1 : D + 2], psum_cm[:, 2 * d + 1 : 2 * d + 2]
                )
                nc.sync.dma_start(out=odv[d], in_=osb[:])

    nc.compile()
    return nc


_NC_CACHE = {}


def _get_nc(n_loc, n_cores):
    key = (n_loc, n_cores)
    if key not in _NC_CACHE:
        _NC_CACHE[key] = _build_nc(n_loc, n_cores)
    return _NC_CACHE[key]


def host_epilogue(stats_g: np.ndarray, md: np.ndarray) -> np.float32:
    """Final scalar loss from global stats ([20, 65]) and md sums ([20]).

    Pure numpy float32; mirrors the reference formula exactly (including the
    fp32 exp overflow -> inf behavior).
    """
    f32 = np.float32
    sums = stats_g[:, :D].astype(f32)
    counts = stats_g[:, D].astype(f32)
    maxc = np.maximum(counts, f32(1.0))
    centers = sums / maxc[:, None]
    m = (md.astype(f32) / maxc).astype(f32)

    c1, c2 = counts[:C], counts[C:]
    valid_intra = (c1 > 1.0) & (c2 > 1.0)
    intra = f32(np.sum(np.where(valid_intra, m[:C] + m[C:], f32(0.0)), dtype=f32))

    ctr1, ctr2 = centers[:C], centers[C:]
    diff = ctr1[:, None, :] - ctr2[None, :, :]
    pd = np.sqrt(np.sum(diff * diff, axis=-1, dtype=f32)).astype(f32)
    valid_c = (c1 > 0.0) & (c2 > 0.0)
    w = (valid_c[:, None] & valid_c[None, :]).astype(f32)
    n_valid = f32(np.sum(valid_c.astype(f32), dtype=f32))
    if n_valid > 1.0:
        inter = f32(np.sum(pd * w, dtype=f32) / np.maximum(n_valid * n_valid, f32(1.0)))
    else:
        inter = f32(0.0)

    normalized = f32(intra / (inter + f32(1e-8)))
    if inter > 0.0:
        with np.errstate(over="ignore"):
            loss = f32(np.log1p(np.exp(normalized / f32(10.0), dtype=f32), dtype=f32))
    else:
        loss = intra
    return np.float32(loss)


def kernel(feat1, label1, feat2, label2, _n_cores=N_CORES, _trace=False):
    n = feat1.shape[0]
    n_loc = n // _n_cores
    nc = _get_nc(n_loc, _n_cores)

    in_maps = []
    for i in range(_n_cores):
        s = slice(i * n_loc, (i + 1) * n_loc)
        in_maps.append(
            {
                "feat1": np.ascontiguousarray(feat1[s]),
                "label1": np.ascontiguousarray(label1[s]),
                "feat2": np.ascontiguousarray(feat2[s]),
                "label2": np.ascontiguousarray(label2[s]),
            }
        )

    res = run_bass_kernel_spmd(
        nc, in_maps, core_ids=list(range(_n_cores)), trace=_trace
    )
    outs = [r["out"] for r in res.results]
    stats_g = outs[0][:, : D + 1]
    md = np.sum([o[:, D + 1] for o in outs], axis=0, dtype=np.float32)
    loss = host_epilogue(stats_g, md)
    if _trace:
        kernel.last_exec_time_ns = res.exec_time_ns
    return loss


kernel.last_exec_time_ns = None


# revision 17
# speedup vs baseline: 1.1911x; 1.1109x over previous
"""Trainium2 Bass kernel for nn_Dist_Loss (segment_reduce).

Data-parallel over 8 NeuronCores: each core takes 1/8 of the rows of
feat1/feat2/label1/label2, computes local per-class sums+counts via one-hot
matmuls (PE, PSUM-accumulated), AllGathers the tiny [2*C, D+1] stats to form
global class centers, then computes per-row distances to own-class centers
entirely from SBUF-resident features (single HBM pass).  Per-class distance
sums are reduced on-device; the final scalar loss formula (tiny, O(C^2 * D))
runs on the host in numpy float32, which reproduces the reference's fp32
overflow semantics (the loss is +inf for the reference inputs).

Features and one-hots are held in bf16 on-chip (one-hots are exact in bf16;
matmuls accumulate fp32 in PSUM) so the PE avoids the 2-pass fp32 LOW_HIGH
matmul mode and LDWEIGHTS uses fast weight load.

Layout: sample s = p*NCOL + n  (p = SBUF partition, n = sample-column).
Per group of GROUP=8 sample-columns (1024 samples):
  - segment sums:  one matmul  lhsT=O_g [128, 80], rhs=feat_g [128, 512]
    -> psum [80, 512]; the 8 diagonal [10, 64] blocks hold valid partial
    sums; the diagonal is folded with a masked matmul afterwards.
  - distances:     per double-group [128, 1024] PSUM: a -I matmul writes
    -feat, a gather matmul (lhsT = PE-transposed one-hot [80, 128],
    rhs = block-diag centers [80, 512]) accumulates +center[label];
    ACT squares, DVE reduces each 64-block -> per-row d^2.

Class index convention: rows 0..9 = domain 1, rows 10..19 = domain 2.
"""

import numpy as np

try:
    import concourse.bass as bass
except ImportError:  # pragma: no cover - fallback when PYTHONPATH is missing
    import sys

    sys.path.insert(0, "/opt/trn_rl_repo")
    import concourse.bass as bass

import concourse.bacc as bacc
import concourse.mybir as mybir
from concourse import tile
from concourse.bass_utils import run_bass_kernel_spmd

F32 = mybir.dt.float32
BF16 = mybir.dt.bfloat16
I32 = mybir.dt.int32
ALU = mybir.AluOpType
AF = mybir.ActivationFunctionType
AX = mybir.AxisListType

N_CORES = 8
N_GLOBAL = 262144
D = 64
C = 10
P = 128
GROUP = 8    # sample-columns per matmul ([*, 512] = one PSUM bank row)
GROUP2 = 16  # sample-columns per pass-B psum tile ([128, 1024], 2 banks)


def _build_nc(n_loc: int, n_cores: int):
    """Trace the SPMD kernel for a per-core shard of n_loc rows per domain."""
    assert n_loc % (P * GROUP2) == 0
    ncol = n_loc // P            # sample-columns per partition, per domain
    ngrp = ncol // GROUP
    ngrp2 = ncol // GROUP2
    nchunks = max(1, ncol // 32)  # DMA chunks per domain
    ccols = ncol // nchunks       # sample-columns per chunk
    assert ccols % GROUP2 == 0

    nc = bacc.Bacc(None, num_devices=n_cores)

    feat_in = [
        nc.dram_tensor(f"feat{d + 1}", [n_loc, D], F32, kind="ExternalInput")
        for d in range(2)
    ]
    lab_in = [
        nc.dram_tensor(f"label{d + 1}", [n_loc], I32, kind="ExternalInput")
        for d in range(2)
    ]
    out_dram = nc.dram_tensor("out", [2 * C, D + 2], F32, kind="ExternalOutput")
    ag_in = [nc.dram_tensor(f"ag_in{d}", [C, D + 1], F32) for d in range(2)]
    ag_out = [
        nc.dram_tensor(
            f"ag_out{d}",
            [C * n_cores, D + 1],
            F32,
            addr_space="Shared" if n_cores > 4 else "Local",
        )
        for d in range(2)
    ]

    with tile.TileContext(nc) as tc:
        with (
            tc.tile_pool(name="big", bufs=1) as big,
            tc.tile_pool(name="ring", bufs=6) as ring,
            tc.tile_pool(name="ps", bufs=1, space="PSUM") as pp,
            tc.tile_pool(name="psr", bufs=2, space="PSUM") as psr,
        ):
            # ---------------- constants ----------------
            iota10 = big.tile([P, C], I32, tag="iota10")
            nc.gpsimd.iota(iota10[:], pattern=[[1, C]], base=0, channel_multiplier=0)

            iota_pj = big.tile([P, P], I32, tag="iota_pj")
            nc.gpsimd.iota(
                iota_pj[:], pattern=[[-1, P]], base=0, channel_multiplier=1
            )
            ident = big.tile([P, P], F32, tag="ident")
            nc.vector.tensor_scalar(
                out=ident[:], in0=iota_pj[:], scalar1=0, scalar2=None, op0=ALU.is_equal
            )
            identb = big.tile([P, P], BF16, tag="identb")
            nc.vector.tensor_copy(identb[:], ident[:])
            negi = big.tile([P, P], BF16, tag="negi")
            nc.vector.tensor_scalar(
                out=negi[:],
                in0=iota_pj[:],
                scalar1=0,
                scalar2=-1.0,
                op0=ALU.is_equal,
                op1=ALU.mult,
            )
            ones = big.tile([P, 1], F32, tag="ones")
            nc.vector.memset(ones[:], 1.0)

            # sel0 [10, 80]: sel0[k, 10t+c] = (k == c)
            iota_ki = big.tile([P, 1], I32, tag="iota_ki")
            nc.gpsimd.iota(iota_ki[:], pattern=[[0, 1]], base=0, channel_multiplier=1)
            iota_k = big.tile([P, 1], F32, tag="iota_k")
            nc.vector.tensor_copy(iota_k[:], iota_ki[:])
            itc = big.tile([C, GROUP * C], I32, tag="iota_tc")
            nc.gpsimd.iota(
                itc[:], pattern=[[0, GROUP], [1, C]], base=0, channel_multiplier=0
            )
            sel0 = big.tile([C, GROUP * C], F32, tag="sel0")
            nc.vector.tensor_scalar(
                out=sel0[:], in0=itc[:], scalar1=iota_k[0:C, :], scalar2=None,
                op0=ALU.is_equal,
            )

            # SEL1 [80, 10] = sel0.T, via PE transpose (folds sums diagonal)
            sel1 = big.tile([GROUP * C, C], F32, tag="sel1")
            sel1_ps = psr.tile([GROUP * C, P], F32, tag="gps", name="sel1_ps")
            nc.tensor.transpose(sel1_ps[:, 0:C], sel0[:], ident[0:C, 0:C])
            nc.scalar.copy(out=sel1[:], in_=sel1_ps[:, 0:C])

            # block-diagonal mask [80, 512]: mask[10t+c, 64t'+j] = (t == t')
            bd_a = big.tile([GROUP, GROUP * C], F32, tag="bd_a")
            bd_ai = big.tile([GROUP, GROUP * C], I32, tag="bd_ai")
            nc.gpsimd.iota(
                bd_ai[:], pattern=[[1, GROUP], [0, C]], base=0, channel_multiplier=0
            )
            nc.vector.tensor_scalar(
                out=bd_a[:], in0=bd_ai[:], scalar1=iota_k[0:GROUP, :],
                scalar2=None, op0=ALU.is_equal,
            )
            bd_b = big.tile([GROUP, GROUP * D], F32, tag="bd_b")
            bd_bi = big.tile([GROUP, GROUP * D], I32, tag="bd_bi")
            nc.gpsimd.iota(
                bd_bi[:], pattern=[[1, GROUP], [0, D]], base=0, channel_multiplier=0
            )
            nc.vector.tensor_scalar(
                out=bd_b[:], in0=bd_bi[:], scalar1=iota_k[0:GROUP, :],
                scalar2=None, op0=ALU.is_equal,
            )
            bdmask = big.tile([GROUP * C, GROUP * D], F32, tag="bdmask")
            bd_ps = psr.tile([GROUP * C, GROUP * D], F32, tag="gps", name="bd_ps")
            nc.tensor.matmul(bd_ps[:], bd_a[:], bd_b[:], start=True, stop=True)
            nc.scalar.copy(out=bdmask[:], in_=bd_ps[:])

            # ---------------- label load + one-hot build ----------------
            lab_sb = []
            o_all = []
            ot_all = []
            cnt_pp = []
            for d in range(2):
                lab = big.tile([P, ncol], I32, tag=f"lab{d}")
                nc.sync.dma_start(
                    out=lab[:], in_=lab_in[d][:].rearrange("(p n) -> p n", p=P)
                )
                lab_sb.append(lab)

                oa = big.tile([P, ncol * C], BF16, tag=f"oall{d}")
                nc.vector.tensor_tensor(
                    out=oa[:].rearrange("p (n c) -> p n c", c=C),
                    in0=lab[:].unsqueeze(2).broadcast_to([P, ncol, C]),
                    in1=iota10[:].unsqueeze(1).broadcast_to([P, ncol, C]),
                    op=ALU.is_equal,
                )
                o_all.append(oa)

                # per-partition class counts, folded across partitions by MM
                cp = big.tile([P, C], F32, tag=f"cntpp{d}")
                nc.vector.tensor_reduce(
                    out=cp[:],
                    in_=oa[:].rearrange("p (n c) -> p c n", c=C),
                    axis=AX.X,
                    op=ALU.add,
                )
                cnt_pp.append(cp)

                # transposed one-hots (bf16, exact) for pass-B gathers
                ota = big.tile([GROUP * C, ngrp * P], BF16, tag=f"otall{d}")
                for g in range(ngrp):
                    otp = psr.tile([GROUP * C, P], BF16, tag="gps",
                                   name=f"otp{d}_{g}")
                    nc.tensor.transpose(
                        otp[:],
                        oa[:, g * GROUP * C : (g + 1) * GROUP * C],
                        identb[:],
                    )
                    nc.scalar.copy(out=ota[:, g * P : (g + 1) * P], in_=otp[:])
                ot_all.append(ota)

            # ---------------- feature load (bf16 cast) + segment sums ----
            # psum_cm columns: [counts_d1, md_d1, counts_d2, md_d2]
            psum_cm = pp.tile([C, 4], F32, tag="cm")
            for d in range(2):
                nc.tensor.matmul(
                    psum_cm[:, 2 * d : 2 * d + 1],
                    cnt_pp[d][:],
                    ones[:],
                    start=True,
                    stop=True,
                    skip_group_check=True,
                )

            feat_sb = [[None] * nchunks for _ in range(2)]
            for d in range(2):
                fr = feat_in[d][:].rearrange("(p n) d -> p (n d)", p=P)
                for k in range(nchunks):
                    ft = big.tile([P, ccols * D], BF16, tag=f"feat{d}_{k}",
                                  name=f"feat{d}_{k}")
                    nc.gpsimd.dma_start(
                        out=ft[:], in_=fr[:, k * ccols * D : (k + 1) * ccols * D]
                    )
                    feat_sb[d][k] = ft

            psum_sums = [
                pp.tile([GROUP * C, GROUP * D], F32, tag=f"sums{d}", name=f"sums{d}")
                for d in range(2)
            ]
            for d in range(2):
                for g in range(ngrp):
                    k = (g * GROUP) // ccols
                    c0 = (g * GROUP) % ccols
                    nc.tensor.matmul(
                        psum_sums[d][:],
                        o_all[d][:, g * GROUP * C : (g + 1) * GROUP * C],
                        feat_sb[d][k][:, c0 * D : (c0 + GROUP) * D],
                        start=(g == 0),
                        stop=(g == ngrp - 1),
                    )

            # per-domain: fold diagonal -> AllGather -> global centers.
            # Domain 0's AllGather overlaps domain 1's streaming sums;
            # domain 1's AllGather overlaps domain 0's pass B.
            stats_g = []
            cbd = []
            for d in range(2):
                # fold the block-diagonal: sums[c, j] = sum_t psum[10t+c, 64t+j]
                s_sb = ring.tile([GROUP * C, GROUP * D], F32, tag="d2r",
                                 name=f"ssb{d}")
                nc.scalar.copy(out=s_sb[:], in_=psum_sums[d][:])
                nc.vector.tensor_tensor(
                    out=s_sb[:], in0=s_sb[:], in1=bdmask[:], op=ALU.mult
                )
                fold_ps = pp.tile([C, GROUP * D], F32, tag="foldps",
                                  name=f"foldps{d}")
                nc.tensor.matmul(fold_ps[:], sel1[:], s_sb[:], start=True, stop=True)
                st = big.tile([C, D + 1], F32, tag=f"stats{d}", name=f"stats{d}")
                nc.vector.tensor_reduce(
                    out=st[:, 0:D],
                    in_=fold_ps[:].rearrange("c (t e) -> c e t", t=GROUP),
                    axis=AX.X,
                    op=ALU.add,
                )
                nc.vector.tensor_copy(
                    st[:, D : D + 1], psum_cm[:, 2 * d : 2 * d + 1]
                )

                nc.sync.dma_start(out=ag_in[d][:], in_=st[:])
                nc.gpsimd.collective_compute(
                    "AllGather",
                    ALU.bypass,
                    replica_groups=[list(range(n_cores))],
                    ins=[ag_in[d][:].opt()],
                    outs=[ag_out[d][:].opt()],
                )
                gath = big.tile([C, n_cores * (D + 1)], F32, tag=f"gath{d}",
                                name=f"gath{d}")
                nc.sync.dma_start(
                    out=gath[:].rearrange("c (r e) -> c r e", r=n_cores),
                    in_=ag_out[d][:].rearrange("(r c) e -> c r e", c=C),
                )
                sg = big.tile([C, D + 1], F32, tag=f"statsg{d}", name=f"statsg{d}")
                nc.vector.tensor_reduce(
                    out=sg[:],
                    in_=gath[:].rearrange("c (r e) -> c e r", r=n_cores),
                    axis=AX.X,
                    op=ALU.add,
                )
                stats_g.append(sg)

                maxc = big.tile([C, 1], F32, tag=f"maxc{d}", name=f"maxc{d}")
                nc.vector.tensor_scalar(
                    out=maxc[:], in0=sg[:, D : D + 1], scalar1=1.0,
                    scalar2=None, op0=ALU.max,
                )
                rec = big.tile([C, 1], F32, tag=f"rec{d}", name=f"rec{d}")
                nc.vector.reciprocal(rec[:], maxc[:])
                cen = big.tile([C, D], F32, tag=f"centers{d}", name=f"centers{d}")
                nc.vector.tensor_scalar(
                    out=cen[:], in0=sg[:, 0:D], scalar1=rec[:],
                    scalar2=None, op0=ALU.mult,
                )
                cen_rep = big.tile([C, GROUP * D], F32, tag=f"cenrep{d}",
                                   name=f"cenrep{d}")
                nc.vector.tensor_copy(
                    cen_rep[:].rearrange("c (t e) -> c t e", t=GROUP),
                    cen[:].unsqueeze(1).broadcast_to([C, GROUP, D]),
                )
                # replicate to [80, 512] on PE, then mask to block-diagonal
                cps = psr.tile([GROUP * C, GROUP * D], F32, tag="gps",
                               name=f"cps{d}")
                nc.tensor.matmul(
                    cps[:], sel0[:], cen_rep[:], start=True, stop=True
                )
                cb = big.tile([GROUP * C, GROUP * D], BF16, tag=f"cbd{d}",
                              name=f"cbd{d}")
                nc.vector.tensor_tensor(
                    out=cb[:], in0=cps[:], in1=bdmask[:], op=ALU.mult
                )
                cbd.append(cb)

            # ---------------- pass B: per-row distances ----------------
            d2_all = [big.tile([P, ncol], F32, tag=f"d2all{d}", name=f"d2all{d}")
                      for d in range(2)]
            d_all = [big.tile([P, ncol], BF16, tag=f"dall{d}", name=f"dall{d}")
                     for d in range(2)]
            for d in range(2):
                for g in range(ngrp2):
                    gps = psr.tile([P, GROUP2 * D], F32, tag="gps",
                                   name=f"gps{d}_{g}")
                    k = (g * GROUP2) // ccols
                    c0 = (g * GROUP2) % ccols
                    for h in range(GROUP2 // GROUP):
                        half = gps[:, h * GROUP * D : (h + 1) * GROUP * D]
                        nc.tensor.matmul(
                            half,
                            negi[:],
                            feat_sb[d][k][
                                :, (c0 + h * GROUP) * D : (c0 + (h + 1) * GROUP) * D
                            ],
                            start=True,
                            stop=False,
                        )
                        gg = g * (GROUP2 // GROUP) + h
                        nc.tensor.matmul(
                            half,
                            ot_all[d][:, gg * P : (gg + 1) * P],
                            cbd[d][:],
                            start=False,
                            stop=True,
                        )
                    d2r = ring.tile([P, GROUP2 * D], F32, tag="d2rb",
                                    name=f"d2r{d}_{g}")
                    nc.scalar.activation(d2r[:], gps[:], AF.Square)
                    nc.vector.tensor_reduce(
                        out=d2_all[d][:, g * GROUP2 : (g + 1) * GROUP2],
                        in_=d2r[:].rearrange("p (t e) -> p t e", e=D),
                        axis=AX.X,
                        op=ALU.add,
                    )

                # per-class distance sums: W = O * d, fold n then partitions,
                # pipelined at half-domain granularity
                w_all = big.tile([P, ncol * C], F32, tag=f"wall{d}",
                                 name=f"wall{d}")
                mv = big.tile([P, 4 * C], F32, tag=f"mv{d}", name=f"mv{d}")
                nh = ncol // 4
                for q in range(4):
                    nc.scalar.activation(
                        d_all[d][:, q * nh : (q + 1) * nh],
                        d2_all[d][:, q * nh : (q + 1) * nh],
                        AF.Sqrt,
                    )
                    wv = w_all[:, q * nh * C : (q + 1) * nh * C]
                    nc.gpsimd.tensor_tensor(
                        out=wv.rearrange("p (n c) -> p n c", c=C),
                        in0=o_all[d][:, q * nh * C : (q + 1) * nh * C].rearrange(
                            "p (n c) -> p n c", c=C
                        ),
                        in1=d_all[d][:, q * nh : (q + 1) * nh]
                        .unsqueeze(2)
                        .broadcast_to([P, nh, C]),
                        op=ALU.mult,
                    )
                    nc.vector.tensor_reduce(
                        out=mv[:, q * C : (q + 1) * C],
                        in_=wv.rearrange("p (n c) -> p c n", c=C),
                        axis=AX.X,
                        op=ALU.add,
                    )
                    nc.tensor.matmul(
                        psum_cm[:, 2 * d + 1 : 2 * d + 2],
                        mv[:, q * C : (q + 1) * C],
                        ones[:],
                        start=(q == 0),
                        stop=(q == 3),
                        skip_group_check=True,
                    )

            # ---------------- output ----------------
            odv = out_dram[:].rearrange("(d c) e -> d c e", d=2)
            for d in range(2):
                osb = big.tile([C, D + 2], F32, tag=f"outsb{d}", name=f"outsb{d}")
                nc.vector.tensor_copy(osb[:, 0 : D + 1], stats_g[d][:])
                nc.vector.tensor_copy(
                    osb[:, D + 1 : D + 2], psum_cm[:, 2 * d + 1 : 2 * d + 2]
                )
                nc.sync.dma_start(out=odv[d], in_=osb[:])

    nc.compile()
    return nc


_NC_CACHE = {}


def _get_nc(n_loc, n_cores):
    key = (n_loc, n_cores)
    if key not in _NC_CACHE:
        _NC_CACHE[key] = _build_nc(n_loc, n_cores)
    return _NC_CACHE[key]


def host_epilogue(stats_g: np.ndarray, md: np.ndarray) -> np.float32:
    """Final scalar loss from global stats ([20, 65]) and md sums ([20]).

    Pure numpy float32; mirrors the reference formula exactly (including the
    fp32 exp overflow -> inf behavior).
    """
    f32 = np.float32
    sums = stats_g[:, :D].astype(f32)
    counts = stats_g[:, D].astype(f32)
    maxc = np.maximum(counts, f32(1.0))
    centers = sums / maxc[:, None]
    m = (md.astype(f32) / maxc).astype(f32)

    c1, c2 = counts[:C], counts[C:]
    valid_intra = (c1 > 1.0) & (c2 > 1.0)
    intra = f32(np.sum(np.where(valid_intra, m[:C] + m[C:], f32(0.0)), dtype=f32))

    ctr1, ctr2 = centers[:C], centers[C:]
    diff = ctr1[:, None, :] - ctr2[None, :, :]
    pd = np.sqrt(np.sum(diff * diff, axis=-1, dtype=f32)).astype(f32)
    valid_c = (c1 > 0.0) & (c2 > 0.0)
    w = (valid_c[:, None] & valid_c[None, :]).astype(f32)
    n_valid = f32(np.sum(valid_c.astype(f32), dtype=f32))
    if n_valid > 1.0:
        inter = f32(np.sum(pd * w, dtype=f32) / np.maximum(n_valid * n_valid, f32(1.0)))
    else:
        inter = f32(0.0)

    normalized = f32(intra / (inter + f32(1e-8)))
    if inter > 0.0:
        with np.errstate(over="ignore"):
            loss = f32(np.log1p(np.exp(normalized / f32(10.0), dtype=f32), dtype=f32))
    else:
        loss = intra
    return np.float32(loss)


def kernel(feat1, label1, feat2, label2, _n_cores=N_CORES, _trace=False):
    n = feat1.shape[0]
    n_loc = n // _n_cores
    nc = _get_nc(n_loc, _n_cores)

    in_maps = []
    for i in range(_n_cores):
        s = slice(i * n_loc, (i + 1) * n_loc)
        in_maps.append(
            {
                "feat1": np.ascontiguousarray(feat1[s]),
                "label1": np.ascontiguousarray(label1[s]),
                "feat2": np.ascontiguousarray(feat2[s]),
                "label2": np.ascontiguousarray(label2[s]),
            }
        )

    res = run_bass_kernel_spmd(
        nc, in_maps, core_ids=list(range(_n_cores)), trace=_trace
    )
    outs = [r["out"] for r in res.results]
    stats_g = outs[0][:, : D + 1]
    md = np.sum([o[:, D + 1] for o in outs], axis=0, dtype=np.float32)
    loss = host_epilogue(stats_g, md)
    if _trace:
        kernel.last_exec_time_ns = res.exec_time_ns
    return loss


kernel.last_exec_time_ns = None
